# revision 10
# baseline (speedup 1.0000x reference)
"""Causal single-head attention (B=4, S=2048, E=1024, D=128) on 8 trn2 cores.

Sharding: 2 cores per batch. Each core computes the attention output for
1024 query rows of its batch. To keep one uniform SPMD program while
balancing the causal (triangular) work, the host permutes each batch's
rows per core role and ships a per-core 0/1 mask table:

  role 0: perm = [0:512 | 512:1024 | 1536:2048 | 1024:1536]
  role 1: perm = [512:1024 | 0:512 | 1024:1536 | 1536:2048]

Queries are the permuted positions [0,512) (q-block 0, key extent 1024)
and [1024,1536) (q-block 1, key extent 2048). Both roles then run the
exact same static program; causality (including wasted padded tiles) is
enforced by multiplying exp(scores) with the host-baked mask.

Per-core kernel (flow over transposed scores, fp32r matmuls):
  xT = PE-transpose of x (E on partitions)
  K^T/V^T/Q^T = w.T @ xT accumulated over 8 E-chunks; V re-transposed
  per q-block, per key tile j: st[t,s] = KT_j.T @ QT ; pt = exp(st*scale)
  pt *= mask ; rowacc += pt ; outT[D,s] += V_j.T @ pt
  rowsum via ones-matmul per 128-col chunk; out = transpose(outT) * 1/rowsum
"""

import math

import numpy as np

B, S, E, D = 4, 2048, 1024, 128
P = 128
EC = E // P          # 8 E-chunks
NT = S // P          # 16 key tiles
TB = S // 512        # 4 key blocks of 512
QB_NT = (8, 16)      # key-tile extent per q-block (padded, role-uniform)
N_MASK = QB_NT[0] + QB_NT[1]
SCALE = 1.0 / math.sqrt(D)

MM_DT = None  # set in _build_nc (float32r)


def _role_perm(role):
    a = np.arange
    if role == 0:
        blocks = [a(0, 512), a(512, 1024), a(1536, 2048), a(1024, 1536)]
    else:
        blocks = [a(512, 1024), a(0, 512), a(1024, 1536), a(1536, 2048)]
    return np.concatenate(blocks)


def _role_mask(role):
    """[128, N_MASK*512] u8: concat over (qb, j) of valid(t_pos, s_pos)."""
    perm = _role_perm(role)
    tiles = []
    for qb, qpos0 in ((0, 0), (1, 1024)):
        q_orig = perm[qpos0 : qpos0 + 512]
        for j in range(QB_NT[qb]):
            t_orig = perm[j * P : (j + 1) * P]
            tiles.append((t_orig[:, None] <= q_orig[None, :]).astype(np.uint8))
    return np.concatenate(tiles, axis=1)


def _build_nc():
    global MM_DT
    from contextlib import ExitStack

    import concourse.bass as bass
    import concourse.tile as tile
    from concourse import bacc, masks, mybir

    MM_DT = mybir.dt.float32r
    f32r = mybir.dt.float32r
    f32 = mybir.dt.float32
    u8 = mybir.dt.uint8
    AF = mybir.ActivationFunctionType

    nc = bacc.Bacc("TRN2", target_bir_lowering=False, debug=False)

    xp = nc.dram_tensor("xp", [S, E], f32, kind="ExternalInput")
    w_in = {
        n: nc.dram_tensor(n, [E, D], f32r, kind="ExternalInput")
        for n in ("wq", "wk", "wv")
    }
    b_in = {
        n: nc.dram_tensor(n, [P, 1], f32, kind="ExternalInput")
        for n in ("bq", "bk", "bv")
    }
    mask_in = nc.dram_tensor("mask", [P, N_MASK * 512], u8, kind="ExternalInput")
    o_out = nc.dram_tensor("o", [1024, D], f32, kind="ExternalOutput")

    def mm(out, lhsT, rhs, start, stop):
        nc.tensor.matmul(out, lhsT, rhs, start=start, stop=stop)

    with tile.TileContext(nc) as tc, ExitStack() as ctx:
        consts = ctx.enter_context(tc.tile_pool(name="consts", bufs=1))
        xn_pool = ctx.enter_context(tc.tile_pool(name="xn", bufs=8))
        xt_pool = ctx.enter_context(tc.tile_pool(name="xt", bufs=2))
        sb_pool = ctx.enter_context(tc.tile_pool(name="sb", bufs=2))
        pt_pool = ctx.enter_context(tc.tile_pool(name="pt", bufs=3))
        out_pool = ctx.enter_context(tc.tile_pool(name="outp", bufs=2))
        tr_psum = ctx.enter_context(tc.tile_pool(name="trp", bufs=2, space="PSUM"))
        proj_psum = ctx.enter_context(tc.tile_pool(name="pjp", bufs=2, space="PSUM"))
        st_psum = ctx.enter_context(tc.tile_pool(name="stp", bufs=2, space="PSUM"))
        ot_psum = ctx.enter_context(tc.tile_pool(name="otp", bufs=1, space="PSUM"))
        sm_psum = ctx.enter_context(tc.tile_pool(name="smp", bufs=1, space="PSUM"))

        # ---- constants -------------------------------------------------
        ident = consts.tile([P, P], f32)
        masks.make_identity(nc, ident[:])
        ones = consts.tile([P, 1], f32)
        nc.vector.memset(ones[:], 1.0)
        w_sb = {}
        for n in ("wq", "wk", "wv"):
            w_sb[n] = consts.tile([P, EC, D], f32r, name=f"w_{n}")
            nc.sync.dma_start(
                out=w_sb[n][:], in_=w_in[n].rearrange("(c p) d -> p c d", p=P)
            )
        b_sb = {}
        for n in ("bq", "bk", "bv"):
            b_sb[n] = consts.tile([P, 1], f32, name=f"b_{n}")
            nc.gpsimd.dma_start(out=b_sb[n][:], in_=b_in[n][:, :])
        mask_sb = consts.tile([P, N_MASK * 512], u8)
        nc.sync.dma_start(out=mask_sb[:], in_=mask_in[:, :])

        kt_sb = consts.tile([P, S], f32r)       # K^T [D, t_pos]
        v_sb = consts.tile([P, NT, D], f32r)    # V natural [t_loc, tile, D]
        qt_sb = consts.tile([P, 1024], f32r)    # Q^T [D, 512*qb + s]

        # ---- phase 1: xT, projections ---------------------------------
        for tb in range(TB):
            xn = []
            for tt in range(4):
                t = xn_pool.tile([P, E], f32, tag="xn")
                nc.sync.dma_start(
                    out=t[:], in_=xp[(tb * 4 + tt) * P : (tb * 4 + tt + 1) * P, :]
                )
                xn.append(t)

            xt = xt_pool.tile([P, EC, 512], f32r)
            for e in range(EC):
                tp = tr_psum.tile([P, 512], f32, tag="tr")
                for tt in range(4):
                    nc.tensor.matmul(
                        tp[:, tt * P : (tt + 1) * P],
                        xn[tt][:, e * P : (e + 1) * P],
                        ident[:],
                        is_transpose=True,
                        start=(tt == 0),
                        stop=(tt == 3),
                    )
                if e % 2 == 0:
                    nc.vector.tensor_copy(xt[:, e, :], tp[:])
                else:
                    nc.scalar.copy(xt[:, e, :], tp[:])

            # K^T
            pp = proj_psum.tile([P, 512], f32, tag="pj")
            for e in range(EC):
                mm(pp[:], w_sb["wk"][:, e, :], xt[:, e, :], e == 0, e == EC - 1)
            nc.scalar.activation(
                out=kt_sb[:, tb * 512 : (tb + 1) * 512],
                in_=pp[:],
                func=AF.Identity,
                bias=b_sb["bk"][:],
            )

            # V^T -> V natural
            pp = proj_psum.tile([P, 512], f32, tag="pj")
            for e in range(EC):
                mm(pp[:], w_sb["wv"][:, e, :], xt[:, e, :], e == 0, e == EC - 1)
            vt = sb_pool.tile([P, 512], f32, tag="vt")
            nc.vector.tensor_scalar_add(vt[:], pp[:], b_sb["bv"][:])
            vp = tr_psum.tile([P, 512], f32, tag="tr")
            for tt in range(4):
                nc.tensor.matmul(
                    vp[:, tt * P : (tt + 1) * P],
                    vt[:, tt * P : (tt + 1) * P],
                    ident[:],
                    is_transpose=True,
                    start=(tt == 0),
                    stop=(tt == 3),
                )
            nc.vector.tensor_copy(v_sb[:, tb * 4 : (tb + 1) * 4, :], vp[:])

            # Q^T (q-block 0 lives at pos [0,512) = tb0; q-block 1 at tb2)
            if tb in (0, 2):
                qb = 0 if tb == 0 else 1
                pp = proj_psum.tile([P, 512], f32, tag="pj")
                for e in range(EC):
                    mm(pp[:], w_sb["wq"][:, e, :], xt[:, e, :], e == 0, e == EC - 1)
                nc.scalar.activation(
                    out=qt_sb[:, qb * 512 : (qb + 1) * 512],
                    in_=pp[:],
                    func=AF.Identity,
                    bias=b_sb["bq"][:],
                )

        # ---- phase 2: attention ---------------------------------------
        for qb in (0, 1):
            n_t = QB_NT[qb]
            qt = qt_sb[:, qb * 512 : (qb + 1) * 512]
            ot = ot_psum.tile([P, 512], f32, tag="ot")
            rowacc = sb_pool.tile([P, 512], f32, tag="racc")
            for j in range(n_t):
                st = st_psum.tile([P, 512], f32, tag="st")
                mm(st[:], kt_sb[:, j * P : (j + 1) * P], qt, True, True)
                pt = pt_pool.tile([P, 512], f32r, tag="pt")
                nc.scalar.activation(out=pt[:], in_=st[:], func=AF.Exp, scale=SCALE)
                midx = qb * QB_NT[0] + j
                nc.vector.tensor_mul(
                    pt[:], pt[:], mask_sb[:, midx * 512 : (midx + 1) * 512]
                )
                if j == 0:
                    nc.gpsimd.tensor_copy(rowacc[:], pt[:].bitcast(f32))
                else:
                    nc.gpsimd.tensor_tensor(
                        rowacc[:], rowacc[:], pt[:].bitcast(f32), op=mybir.AluOpType.add
                    )
                mm(ot[:], v_sb[:, j, :], pt[:], j == 0, j == n_t - 1)

            # rowsum per 128-col chunk: [128,1] = rowacc_chunk.T @ ones
            rs = sm_psum.tile([P, 4], f32, tag="sm")
            for c in range(4):
                mm(rs[:, c : c + 1], rowacc[:, c * P : (c + 1) * P], ones[:],
                   c == 0, c == 3)
            rcp = sb_pool.tile([P, 4], f32, tag="rcp")
            nc.vector.reciprocal(rcp[:], rs[:])

            for c in range(4):
                ots = sb_pool.tile([P, P], f32, tag="ots")
                nc.vector.tensor_copy(ots[:], ot[:, c * P : (c + 1) * P])
                on = sm_psum.tile([P, P], f32, tag="sm")
                nc.tensor.matmul(
                    on[:], ots[:], ident[:], is_transpose=True, start=True, stop=True
                )
                fin = out_pool.tile([P, D], f32, tag="fin")
                nc.scalar.activation(
                    out=fin[:], in_=on[:], func=AF.Copy, scale=rcp[:, c : c + 1]
                )
                nc.sync.dma_start(
                    out=o_out[qb * 512 + c * P : qb * 512 + (c + 1) * P, :],
                    in_=fin[:],
                )

    nc.compile()
    return nc


_NC_CACHE = {}


def _get_nc():
    if "nc" not in _NC_CACHE:
        _NC_CACHE["nc"] = _build_nc()
    return _NC_CACHE["nc"]


def _get_runner():
    """Cached PJRT executable (same lowering as bass2jax.run_bass_via_pjrt,
    but the jitted function is built once and reused across calls)."""
    if "runner" in _NC_CACHE:
        return _NC_CACHE["runner"]

    import jax
    import jax.numpy as jnp
    from jax.sharding import Mesh, PartitionSpec
    from jax.experimental.shard_map import shard_map
    from concourse import bass2jax, mybir

    nc = _get_nc()
    bass2jax.install_neuronx_cc_hook()

    partition_name = nc.partition_id_tensor.name if nc.partition_id_tensor else None
    in_names, out_names, out_avals = [], [], []
    for alloc in nc.m.functions[0].allocations:
        if not isinstance(alloc, mybir.MemoryLocationSet):
            continue
        name = alloc.memorylocations[0].name
        if alloc.kind == "ExternalInput":
            if name != partition_name:
                in_names.append(name)
        elif alloc.kind == "ExternalOutput":
            out_names.append(name)
            out_avals.append(
                jax.core.ShapedArray(tuple(alloc.tensor_shape), mybir.dt.np(alloc.dtype))
            )
    n_params = len(in_names)
    n_outs = len(out_names)
    all_names = in_names + out_names
    if partition_name is not None:
        all_names = all_names + [partition_name]

    def _body(*args):
        operands = list(args)
        if partition_name is not None:
            operands.append(bass2jax.partition_id_tensor())
        outs = bass2jax._bass_exec_p.bind(
            *operands,
            out_avals=tuple(out_avals),
            in_names=tuple(all_names),
            out_names=tuple(out_names),
            lowering_input_output_aliases=(),
            sim_require_finite=True,
            sim_require_nnan=True,
            nc=nc,
        )
        return tuple(outs)

    devices = jax.devices()[:8]
    mesh = Mesh(__import__("numpy").asarray(devices), ("core",))
    sharded = jax.jit(
        shard_map(
            _body,
            mesh=mesh,
            in_specs=(PartitionSpec("core"),) * (n_params + n_outs),
            out_specs=(PartitionSpec("core"),) * n_outs,
            check_rep=False,
        ),
        donate_argnums=tuple(range(n_params, n_params + n_outs)),
        keep_unused=True,
    )
    runner = {
        "sharded": sharded,
        "in_names": in_names,
        "out_names": out_names,
        "out_avals": out_avals,
    }
    _NC_CACHE["runner"] = runner
    return runner


def _prep_in_concat(x, wq, bq, wk, bk, wv, bv):
    """Per-core in_maps, concatenated along axis 0 for shard_map."""
    x = np.asarray(x, dtype=np.float32)
    w = {
        "wq": np.asarray(wq, np.float32),
        "wk": np.asarray(wk, np.float32),
        "wv": np.asarray(wv, np.float32),
        "bq": np.asarray(bq, np.float32).reshape(P, 1),
        "bk": np.asarray(bk, np.float32).reshape(P, 1),
        "bv": np.asarray(bv, np.float32).reshape(P, 1),
    }
    if "perm" not in _NC_CACHE:
        _NC_CACHE["perm"] = [_role_perm(0), _role_perm(1)]
        _NC_CACHE["mask"] = [
            np.ascontiguousarray(_role_mask(0)),
            np.ascontiguousarray(_role_mask(1)),
        ]
    perms, msks = _NC_CACHE["perm"], _NC_CACHE["mask"]

    runner = _get_runner()
    concat = {}
    concat["xp"] = np.concatenate(
        [x[c // 2][perms[c % 2]] for c in range(8)], axis=0
    )
    concat["mask"] = np.concatenate([msks[c % 2] for c in range(8)], axis=0)
    for n, v in w.items():
        concat[n] = np.concatenate([v] * 8, axis=0)
    return [concat[n] for n in runner["in_names"]]


def _run_concat(concat_in):
    runner = _get_runner()
    zeros = [
        np.zeros((8 * a.shape[0], *a.shape[1:]), a.dtype) for a in runner["out_avals"]
    ]
    out_arrs = runner["sharded"](*concat_in, *zeros)
    o = np.asarray(out_arrs[runner["out_names"].index("o")]).reshape(8, 1024, D)
    return o


def _assemble(o_cores):
    perms = _NC_CACHE["perm"]
    out = np.empty((B, S, D), dtype=np.float32)
    for c in range(8):
        b, role = divmod(c, 2)
        perm = perms[role]
        out[b, perm[0:512]] = o_cores[c][0:512]
        out[b, perm[1024:1536]] = o_cores[c][512:1024]
    return out


def kernel(x, wq, bq, wk, bk, wv, bv):
    concat_in = _prep_in_concat(x, wq, bq, wk, bk, wv, bv)
    o = _run_concat(concat_in)
    return _assemble(o)


def bench(x, wq, bq, wk, bk, wv, bv, iters=20):
    """Per-launch wall time with device-resident inputs (upper bound on HW exec)."""
    import time

    import jax

    runner = _get_runner()
    concat_in = _prep_in_concat(x, wq, bq, wk, bk, wv, bv)
    dev_in = [jax.device_put(a) for a in concat_in]
    for a in dev_in:
        a.block_until_ready()
    times = []
    for _ in range(iters):
        zeros = [
            np.zeros((8 * a.shape[0], *a.shape[1:]), a.dtype)
            for a in runner["out_avals"]
        ]
        t0 = time.perf_counter()
        out = runner["sharded"](*dev_in, *zeros)
        for a in out:
            a.block_until_ready()
        times.append(time.perf_counter() - t0)
    return times


# revision 21
# speedup vs baseline: 1519.7446x; 1519.7446x over previous
"""Causal single-head attention (B=4, S=2048, E=1024, D=128) on 8 trn2 cores.

Sharding: 2 cores per batch. Each core computes the attention output for
1024 query rows of its batch. To keep one uniform SPMD program while
balancing the causal (triangular) work, the host permutes each batch's
rows per core role and ships a per-core 0/1 mask table:

  role 0: perm = [0:512 | 512:1024 | 1536:2048 | 1024:1536]
  role 1: perm = [512:1024 | 0:512 | 1024:1536 | 1536:2048]

Queries are the permuted positions [0,512) (q-block 0, key extent 1024)
and [1024,1536) (q-block 1, key extent 2048). Both roles then run the
exact same static program; causality (including wasted padded tiles) is
enforced by multiplying exp(scores) with the host-baked mask.

Per-core kernel (flow over transposed scores, fp32r matmuls):
  xT = PE-transpose of x (E on partitions)
  K^T/V^T/Q^T = w.T @ xT accumulated over 8 E-chunks; V re-transposed
  per q-block, per key tile j: st[t,s] = KT_j.T @ QT ; pt = exp(st*scale)
  pt *= mask ; rowacc += pt ; outT[D,s] += V_j.T @ pt
  rowsum via ones-matmul per 128-col chunk; out = transpose(outT) * 1/rowsum
"""

import math

import numpy as np

B, S, E, D = 4, 2048, 1024, 128
P = 128
EC = E // P          # 8 E-chunks
NT = S // P          # 16 key tiles
TB = S // 512        # 4 key blocks of 512
QB_NT = (8, 16)      # key-tile extent per q-block (padded, role-uniform)
N_MASK = QB_NT[0] + QB_NT[1]
SCALE = 1.0 / math.sqrt(D)

MM_DT = None  # set in _build_nc (float32r)


def _role_perm(role):
    a = np.arange
    if role == 0:
        blocks = [a(0, 512), a(512, 1024), a(1536, 2048), a(1024, 1536)]
    else:
        blocks = [a(512, 1024), a(0, 512), a(1024, 1536), a(1536, 2048)]
    return np.concatenate(blocks)


def _role_mask(role):
    """[128, N_MASK*512] u8: concat over (qb, j) of valid(t_pos, s_pos)."""
    perm = _role_perm(role)
    tiles = []
    for qb, qpos0 in ((0, 0), (1, 1024)):
        q_orig = perm[qpos0 : qpos0 + 512]
        for j in range(QB_NT[qb]):
            t_orig = perm[j * P : (j + 1) * P]
            tiles.append((t_orig[:, None] <= q_orig[None, :]).astype(np.uint8))
    return np.concatenate(tiles, axis=1)


def _build_nc():
    global MM_DT
    from contextlib import ExitStack

    import concourse.bass as bass
    import concourse.tile as tile
    from concourse import bacc, masks, mybir

    MM_DT = mybir.dt.float32r
    f32r = mybir.dt.float32r
    f32 = mybir.dt.float32
    u8 = mybir.dt.uint8
    AF = mybir.ActivationFunctionType

    nc = bacc.Bacc("TRN2", target_bir_lowering=False, debug=False)

    xp = nc.dram_tensor("xp", [S, E], f32r, kind="ExternalInput")
    w_in = {
        n: nc.dram_tensor(n, [E, D], f32r, kind="ExternalInput")
        for n in ("wq", "wk", "wv")
    }
    b_in = {
        n: nc.dram_tensor(n, [P, 1], f32, kind="ExternalInput")
        for n in ("bq", "bk", "bv")
    }
    tpos_in = nc.dram_tensor("tpos", [P, NT], f32, kind="ExternalInput")
    ident_in = nc.dram_tensor("ident", [P, P], f32r, kind="ExternalInput")
    ones_in = nc.dram_tensor("ones", [P, 1], f32r, kind="ExternalInput")
    qpos_in = nc.dram_tensor("qpos", [1, 1024], f32, kind="ExternalInput")
    ot_out = nc.dram_tensor("ot", [P, 1024], f32, kind="ExternalOutput")
    rs_out = nc.dram_tensor("rs", [1, 1024], f32, kind="ExternalOutput")

    def mm(out, lhsT, rhs, start, stop):
        nc.tensor.matmul(out, lhsT, rhs, start=start, stop=stop)

    with tile.TileContext(nc) as tc, ExitStack() as ctx:
        consts = ctx.enter_context(tc.tile_pool(name="consts", bufs=1))
        xn_pool = ctx.enter_context(tc.tile_pool(name="xn", bufs=8))
        xt_pool = ctx.enter_context(tc.tile_pool(name="xt", bufs=16))
        sb_pool = ctx.enter_context(tc.tile_pool(name="sb", bufs=2))
        pt_pool = ctx.enter_context(tc.tile_pool(name="pt", bufs=6))
        out_pool = ctx.enter_context(tc.tile_pool(name="outp", bufs=1))
        tr_psum = ctx.enter_context(tc.tile_pool(name="trp", bufs=2, space="PSUM"))
        st_psum = ctx.enter_context(tc.tile_pool(name="stp", bufs=2, space="PSUM"))
        proj_psum = ctx.enter_context(tc.tile_pool(name="pjp", bufs=3, space="PSUM"))
        sm_psum = ctx.enter_context(tc.tile_pool(name="smp", bufs=1, space="PSUM"))

        # identity first: the very first transposes need it
        ident_t = consts.tile([P, P], f32r, name="ident_t")
        nc.sync.dma_start(out=ident_t[:], in_=ident_in[:, :])
        ident = ident_t[:]

        # stage the first T-block's x rows before anything else so PE can
        # start transposing as early as possible
        xn_tiles = {}
        dmae = [nc.sync, nc.scalar]
        for tt in range(4):
            t = xn_pool.tile([P, E], f32r, tag="xn", name=f"xn_{tt}")
            dmae[tt % 2].dma_start(out=t[:], in_=xp[tt * P : (tt + 1) * P, :])
            xn_tiles[tt] = t

        ones = consts.tile([P, 1], f32r)
        nc.gpsimd.dma_start(out=ones[:], in_=ones_in[:, :])
        w_sb = {}
        for i, n in enumerate(("wk", "wv", "wq")):
            w_sb[n] = consts.tile([P, EC, D], f32r, name=f"w_{n}")
            dmae[i % 2].dma_start(
                out=w_sb[n][:], in_=w_in[n].rearrange("(c p) d -> p c d", p=P)
            )
        b_sb = {}
        for n in ("bq", "bk", "bv"):
            b_sb[n] = consts.tile([P, 1], f32, name=f"b_{n}")
            nc.gpsimd.dma_start(out=b_sb[n][:], in_=b_in[n][:, :])

        kt_tiles = []   # per-tb K^T [D, 512]
        v_tiles = []    # per-tb V natural [t_loc, 4, D]
        qt_tiles = {}   # per-qb Q^T [D, 512]
        tpos_sb = consts.tile([P, NT], f32)
        nc.gpsimd.dma_start(out=tpos_sb[:], in_=tpos_in[:, :])
        qpos_sb = consts.tile([P, 1024], f32)
        nc.sync.dma_start(
            out=qpos_sb[:],
            in_=bass.AP(
                tensor=qpos_in.ap().tensor,
                offset=0,
                ap=[[0, P], [1, 1024]],
            ),
        )

        # ---- phase 1: xT, projections ---------------------------------
        for tb in range(TB):
            xn = []
            for tt in range(4):
                g = tb * 4 + tt
                if g in xn_tiles:
                    t = xn_tiles[g]
                else:
                    t = xn_pool.tile([P, E], f32r, tag="xn", name=f"xn_{g}")
                    dmae[g % 2].dma_start(out=t[:], in_=xp[g * P : (g + 1) * P, :])
                xn.append(t)

            xt = []
            for e in range(EC):
                tp = tr_psum.tile([P, 512], f32r, tag="tr")
                for tt in range(4):
                    nc.tensor.matmul(
                        tp[:, tt * P : (tt + 1) * P],
                        xn[tt][:, e * P : (e + 1) * P],
                        ident,
                        is_transpose=True,
                        start=(tt == 0),
                        stop=(tt == 3),
                    )
                xte = xt_pool.tile([P, 512], f32r, tag="xte", name=f"xt_{tb}_{e}")
                if e % 2 == 0:
                    nc.vector.tensor_copy(xte[:], tp[:])
                else:
                    nc.scalar.copy(xte[:], tp[:])
                xt.append(xte)

            # K^T
            pp = proj_psum.tile([P, 512], f32, tag="pj")
            for e in range(EC):
                mm(pp[:], w_sb["wk"][:, e, :], xt[e][:], e == 0, e == EC - 1)
            kt = consts.tile([P, 512], f32r, name=f"kt_{tb}")
            nc.scalar.activation(
                out=kt[:], in_=pp[:], func=AF.Identity, bias=b_sb["bk"][:]
            )
            kt_tiles.append(kt)

            # V^T -> V natural
            pp = proj_psum.tile([P, 512], f32, tag="pj")
            for e in range(EC):
                mm(pp[:], w_sb["wv"][:, e, :], xt[e][:], e == 0, e == EC - 1)
            vt = sb_pool.tile([P, 512], f32r, tag="vt")
            nc.scalar.activation(
                out=vt[:], in_=pp[:], func=AF.Identity, bias=b_sb["bv"][:]
            )
            vp = tr_psum.tile([P, 512], f32r, tag="tr")
            for tt in range(4):
                nc.tensor.matmul(
                    vp[:, tt * P : (tt + 1) * P],
                    vt[:, tt * P : (tt + 1) * P],
                    ident,
                    is_transpose=True,
                    start=(tt == 0),
                    stop=(tt == 3),
                )
            v = consts.tile([P, 4, D], f32r, name=f"v_{tb}")
            nc.vector.tensor_copy(v[:], vp[:])
            v_tiles.append(v)

            # Q^T (q-block 0 lives at pos [0,512) = tb0; q-block 1 at tb2)
            if tb in (0, 2):
                qb = 0 if tb == 0 else 1
                pp = proj_psum.tile([P, 512], f32, tag="pj")
                for e in range(EC):
                    mm(pp[:], w_sb["wq"][:, e, :], xt[e][:], e == 0, e == EC - 1)
                qt = consts.tile([P, 512], f32r, name=f"qt_{qb}")
                nc.scalar.activation(
                    out=qt[:], in_=pp[:], func=AF.Identity, bias=b_sb["bq"][:]
                )
                qt_tiles[qb] = qt

        # ---- phase 2: attention ---------------------------------------
        ot_sb = out_pool.tile([P, 1024], f32)
        rs_sb = out_pool.tile([1, 1024], f32)
        for qb in (0, 1):
            n_t = QB_NT[qb]
            qt = qt_tiles[qb]
            qpos = qpos_sb[:, qb * 512 : (qb + 1) * 512]
            ot = proj_psum.tile([P, 512], f32, tag="pj")
            rs = sm_psum.tile([1, 512], f32, tag="sm")
            for j in range(n_t):
                st = st_psum.tile([P, 512], f32, tag="st")
                mm(st[:], kt_tiles[j // 4][:, (j % 4) * P : (j % 4 + 1) * P],
                   qt[:], True, True)
                pt = pt_pool.tile([P, 512], f32r, tag="pt")
                nc.scalar.activation(out=pt[:], in_=st[:], func=AF.Exp, scale=SCALE)
                nc.vector.scalar_tensor_tensor(
                    out=pt[:],
                    in0=qpos,
                    scalar=tpos_sb[:, j : j + 1],
                    in1=pt[:],
                    op0=mybir.AluOpType.is_ge,
                    op1=mybir.AluOpType.mult,
                )
                mm(ot[:], v_tiles[j // 4][:, j % 4, :], pt[:], j == 0, j == n_t - 1)
                mm(rs[:], ones[:], pt[:], j == 0, j == n_t - 1)

            nc.vector.tensor_copy(rs_sb[0:1, qb * 512 : (qb + 1) * 512], rs[:])
            nc.vector.tensor_copy(ot_sb[:, qb * 512 : (qb + 1) * 512], ot[:])
            nc.sync.dma_start(
                out=ot_out[:, qb * 512 : (qb + 1) * 512],
                in_=ot_sb[:, qb * 512 : (qb + 1) * 512],
            )
            nc.scalar.dma_start(
                out=rs_out[:, qb * 512 : (qb + 1) * 512],
                in_=rs_sb[0:1, qb * 512 : (qb + 1) * 512],
            )

    nc.compile()
    return nc


_NC_CACHE = {}


def _get_nc():
    if "nc" not in _NC_CACHE:
        _NC_CACHE["nc"] = _build_nc()
    return _NC_CACHE["nc"]


def _get_runner():
    """Cached PJRT executable (same lowering as bass2jax.run_bass_via_pjrt,
    but the jitted function is built once and reused across calls)."""
    if "runner" in _NC_CACHE:
        return _NC_CACHE["runner"]

    import jax
    import jax.numpy as jnp
    from jax.sharding import Mesh, PartitionSpec
    from jax.experimental.shard_map import shard_map
    from concourse import bass2jax, mybir

    nc = _get_nc()
    bass2jax.install_neuronx_cc_hook()

    partition_name = nc.partition_id_tensor.name if nc.partition_id_tensor else None
    in_names, out_names, out_avals = [], [], []
    for alloc in nc.m.functions[0].allocations:
        if not isinstance(alloc, mybir.MemoryLocationSet):
            continue
        name = alloc.memorylocations[0].name
        if alloc.kind == "ExternalInput":
            if name != partition_name:
                in_names.append(name)
        elif alloc.kind == "ExternalOutput":
            out_names.append(name)
            out_avals.append(
                jax.core.ShapedArray(tuple(alloc.tensor_shape), mybir.dt.np(alloc.dtype))
            )
    n_params = len(in_names)
    n_outs = len(out_names)
    all_names = in_names + out_names
    if partition_name is not None:
        all_names = all_names + [partition_name]

    def _body(*args):
        operands = list(args)
        if partition_name is not None:
            operands.append(bass2jax.partition_id_tensor())
        outs = bass2jax._bass_exec_p.bind(
            *operands,
            out_avals=tuple(out_avals),
            in_names=tuple(all_names),
            out_names=tuple(out_names),
            lowering_input_output_aliases=(),
            sim_require_finite=True,
            sim_require_nnan=True,
            nc=nc,
        )
        return tuple(outs)

    devices = jax.devices()[:8]
    mesh = Mesh(__import__("numpy").asarray(devices), ("core",))
    sharded = jax.jit(
        shard_map(
            _body,
            mesh=mesh,
            in_specs=(PartitionSpec("core"),) * (n_params + n_outs),
            out_specs=(PartitionSpec("core"),) * n_outs,
            check_rep=False,
        ),
        donate_argnums=tuple(range(n_params, n_params + n_outs)),
        keep_unused=True,
    )
    runner = {
        "sharded": sharded,
        "in_names": in_names,
        "out_names": out_names,
        "out_avals": out_avals,
    }
    _NC_CACHE["runner"] = runner
    return runner


def _prep_in_concat(x, wq, bq, wk, bk, wv, bv):
    """Per-core in_maps, concatenated along axis 0 for shard_map."""
    x = np.asarray(x, dtype=np.float32)
    w = {
        "wq": np.asarray(wq, np.float32),
        "wk": np.asarray(wk, np.float32),
        "wv": np.asarray(wv, np.float32),
        "bq": np.asarray(bq, np.float32).reshape(P, 1),
        "bk": np.asarray(bk, np.float32).reshape(P, 1),
        "bv": np.asarray(bv, np.float32).reshape(P, 1),
    }
    if "perm" not in _NC_CACHE:
        _NC_CACHE["perm"] = [_role_perm(0), _role_perm(1)]
        tp, qp = [], []
        for role in (0, 1):
            perm = _NC_CACHE["perm"][role]
            tp.append(
                np.ascontiguousarray(
                    perm.reshape(NT, P).T.astype(np.float32)
                )
            )
            qp.append(
                np.ascontiguousarray(
                    np.concatenate([perm[0:512], perm[1024:1536]])
                    .astype(np.float32)
                    .reshape(1, 1024)
                )
            )
        _NC_CACHE["tpos"] = tp
        _NC_CACHE["qpos"] = qp
    perms = _NC_CACHE["perm"]

    runner = _get_runner()
    concat = {}
    concat["xp"] = np.concatenate(
        [x[c // 2][perms[c % 2]] for c in range(8)], axis=0
    )
    concat["tpos"] = np.concatenate([_NC_CACHE["tpos"][c % 2] for c in range(8)], axis=0)
    concat["ident"] = np.concatenate([np.eye(P, dtype=np.float32)] * 8, axis=0)
    concat["ones"] = np.ones((8 * P, 1), dtype=np.float32)
    concat["qpos"] = np.concatenate([_NC_CACHE["qpos"][c % 2] for c in range(8)], axis=0)
    for n, v in w.items():
        concat[n] = np.concatenate([v] * 8, axis=0)
    return [concat[n] for n in runner["in_names"]]


def _run_concat(concat_in):
    runner = _get_runner()
    zeros = [
        np.zeros((8 * a.shape[0], *a.shape[1:]), a.dtype) for a in runner["out_avals"]
    ]
    out_arrs = runner["sharded"](*concat_in, *zeros)
    ot = np.asarray(out_arrs[runner["out_names"].index("ot")]).reshape(8, P, 1024)
    rs = np.asarray(out_arrs[runner["out_names"].index("rs")]).reshape(8, 1024)
    return ot, rs


def _assemble(ot, rs):
    perms = _NC_CACHE["perm"]
    out = np.empty((B, S, D), dtype=np.float32)
    for c in range(8):
        b, role = divmod(c, 2)
        perm = perms[role]
        for qb, qpos0 in ((0, 0), (1, 1024)):
            otT = ot[c][:, qb * 512 : (qb + 1) * 512]          # [D, 512]
            rsq = rs[c][qb * 512 : (qb + 1) * 512]             # [512]
            out[b, perm[qpos0 : qpos0 + 512]] = (otT / rsq[None, :]).T
    return out


def kernel(x, wq, bq, wk, bk, wv, bv):
    concat_in = _prep_in_concat(x, wq, bq, wk, bk, wv, bv)
    ot, rs = _run_concat(concat_in)
    return _assemble(ot, rs)


def bench(x, wq, bq, wk, bk, wv, bv, iters=20):
    """Per-launch wall time with device-resident inputs (upper bound on HW exec)."""
    import time

    import jax

    runner = _get_runner()
    concat_in = _prep_in_concat(x, wq, bq, wk, bk, wv, bv)
    dev_in = [jax.device_put(a) for a in concat_in]
    for a in dev_in:
        a.block_until_ready()
    times = []
    for _ in range(iters):
        zeros = [
            np.zeros((8 * a.shape[0], *a.shape[1:]), a.dtype)
            for a in runner["out_avals"]
        ]
        t0 = time.perf_counter()
        out = runner["sharded"](*dev_in, *zeros)
        for a in out:
            a.block_until_ready()
        times.append(time.perf_counter() - t0)
    return times


def bench_chain(x, wq, bq, wk, bk, wv, bv, ks=(2, 12), reps=6):
    """Marginal device time per kernel launch: chain k sequential launches
    inside one jit (data-dependent via the mask input), compare wall."""
    import time

    import jax
    import jax.numpy as jnp
    from jax.sharding import Mesh, PartitionSpec
    from jax.experimental.shard_map import shard_map
    from concourse import bass2jax

    runner = _get_runner()
    nc = _get_nc()
    partition_name = nc.partition_id_tensor.name if nc.partition_id_tensor else None
    in_names = runner["in_names"]
    out_names = runner["out_names"]
    out_avals = runner["out_avals"]
    all_names = in_names + out_names + ([partition_name] if partition_name else [])
    mask_idx = in_names.index("mask")

    concat_in = _prep_in_concat(x, wq, bq, wk, bk, wv, bv)
    dev_in = [jax.device_put(a) for a in concat_in]
    for a in dev_in:
        a.block_until_ready()

    import numpy as _np

    def make_fn(k):
        def _body(*args):
            ins = list(args[: len(in_names)])
            zero_sets = args[len(in_names) :]
            o = None
            outs = None
            for i in range(k):
                cur = list(ins)
                if o is not None:
                    bump = (o[0:1, 0:1] != o[0:1, 0:1]).astype(jnp.uint8)
                    cur[mask_idx] = cur[mask_idx] | bump
                operands = cur + list(zero_sets[i * len(out_names) : (i + 1) * len(out_names)])
                if partition_name is not None:
                    operands.append(bass2jax.partition_id_tensor())
                outs = bass2jax._bass_exec_p.bind(
                    *operands,
                    out_avals=tuple(out_avals),
                    in_names=tuple(all_names),
                    out_names=tuple(out_names),
                    lowering_input_output_aliases=(),
                    sim_require_finite=True,
                    sim_require_nnan=True,
                    nc=nc,
                )
                o = outs[0]
            return tuple(outs)

        n_z = k * len(out_names)
        devices = jax.devices()[:8]
        mesh = Mesh(_np.asarray(devices), ("core",))
        return jax.jit(
            shard_map(
                _body,
                mesh=mesh,
                in_specs=(PartitionSpec("core"),) * (len(in_names) + n_z),
                out_specs=(PartitionSpec("core"),) * len(out_names),
                check_rep=False,
            ),
            donate_argnums=tuple(range(len(in_names), len(in_names) + n_z)),
            keep_unused=True,
        )

    results = {}
    for k in ks:
        fn = make_fn(k)
        walls = []
        for _ in range(reps):
            zeros = [
                _np.zeros((8 * a.shape[0], *a.shape[1:]), a.dtype)
                for _ in range(k)
                for a in out_avals
            ]
            t0 = time.perf_counter()
            out = fn(*dev_in, *zeros)
            for a in out:
                a.block_until_ready()
            walls.append(time.perf_counter() - t0)
        results[k] = min(walls)
    k0, k1 = ks
    per_launch = (results[k1] - results[k0]) / (k1 - k0)
    return per_launch, results


# revision 28
# speedup vs baseline: 1584.4985x; 1.0426x over previous
"""Causal single-head attention (B=4, S=2048, E=1024, D=128) on 8 trn2 cores.

Sharding: 2 cores per batch. Each core computes the attention output for
1024 query rows of its batch. To keep one uniform SPMD program while
balancing the causal (triangular) work, the host permutes each batch's
rows per core role and ships a per-core 0/1 mask table:

  role 0: perm = [0:512 | 512:1024 | 1536:2048 | 1024:1536]
  role 1: perm = [512:1024 | 0:512 | 1024:1536 | 1536:2048]

Queries are the permuted positions [0,512) (q-block 0, key extent 1024)
and [1024,1536) (q-block 1, key extent 2048). Both roles then run the
exact same static program; causality (including wasted padded tiles) is
enforced by multiplying exp(scores) with the host-baked mask.

Per-core kernel (flow over transposed scores, fp32r matmuls):
  xT = PE-transpose of x (E on partitions)
  K^T/V^T/Q^T = w.T @ xT accumulated over 8 E-chunks; V re-transposed
  per q-block, per key tile j: st[t,s] = KT_j.T @ QT ; pt = exp(st*scale)
  pt *= mask ; rowacc += pt ; outT[D,s] += V_j.T @ pt
  rowsum via ones-matmul per 128-col chunk; out = transpose(outT) * 1/rowsum
"""

import math

import numpy as np

B, S, E, D = 4, 2048, 1024, 128
P = 128
EC = E // P          # 8 E-chunks
NT = S // P          # 16 key tiles
TB = S // 512        # 4 key blocks of 512
QB_NT = (8, 16)      # key-tile extent per q-block (padded, role-uniform)
N_MASK = QB_NT[0] + QB_NT[1]
SCALE = 1.0 / math.sqrt(D)

MM_DT = None  # set in _build_nc (float32r)


def _role_perm(role):
    a = np.arange
    if role == 0:
        blocks = [a(0, 512), a(512, 1024), a(1536, 2048), a(1024, 1536)]
    else:
        blocks = [a(512, 1024), a(0, 512), a(1024, 1536), a(1536, 2048)]
    return np.concatenate(blocks)


def _role_mask(role):
    """[128, N_MASK*512] u8: concat over (qb, j) of valid(t_pos, s_pos)."""
    perm = _role_perm(role)
    tiles = []
    for qb, qpos0 in ((0, 0), (1, 1024)):
        q_orig = perm[qpos0 : qpos0 + 512]
        for j in range(QB_NT[qb]):
            t_orig = perm[j * P : (j + 1) * P]
            tiles.append((t_orig[:, None] <= q_orig[None, :]).astype(np.uint8))
    return np.concatenate(tiles, axis=1)


def _build_nc():
    global MM_DT
    from contextlib import ExitStack

    import concourse.bass as bass
    import concourse.tile as tile
    from concourse import bacc, masks, mybir

    MM_DT = mybir.dt.float32r
    f32r = mybir.dt.float32r
    f32 = mybir.dt.float32
    u8 = mybir.dt.uint8
    AF = mybir.ActivationFunctionType

    nc = bacc.Bacc("TRN2", target_bir_lowering=False, debug=False)

    xp = nc.dram_tensor("xp", [S, E], f32r, kind="ExternalInput")
    w_in = {
        n: nc.dram_tensor(n, [E, D], f32r, kind="ExternalInput")
        for n in ("wq", "wk", "wv")
    }
    b_in = {
        n: nc.dram_tensor(n, [P, 1], f32, kind="ExternalInput")
        for n in ("bq", "bk", "bv")
    }
    tpos_in = nc.dram_tensor("tpos", [P, NT], f32, kind="ExternalInput")
    ident_in = nc.dram_tensor("ident", [P, P], f32r, kind="ExternalInput")
    ones_in = nc.dram_tensor("ones", [P, 1], f32r, kind="ExternalInput")
    qpos_in = nc.dram_tensor("qpos", [1, 1024], f32, kind="ExternalInput")
    ot_out = nc.dram_tensor("ot", [P, 1024], f32, kind="ExternalOutput")
    rs_out = nc.dram_tensor("rs", [1, 1024], f32, kind="ExternalOutput")

    def mm(out, lhsT, rhs, start, stop):
        nc.tensor.matmul(out, lhsT, rhs, start=start, stop=stop)

    with tile.TileContext(nc) as tc, ExitStack() as ctx:
        consts = ctx.enter_context(tc.tile_pool(name="consts", bufs=1))
        xn_pool = ctx.enter_context(tc.tile_pool(name="xn", bufs=16))
        xt_pool = ctx.enter_context(tc.tile_pool(name="xt", bufs=16))
        sb_pool = ctx.enter_context(tc.tile_pool(name="sb", bufs=2))
        pt_pool = ctx.enter_context(tc.tile_pool(name="pt", bufs=6))
        out_pool = ctx.enter_context(tc.tile_pool(name="outp", bufs=1))
        tr_psum = ctx.enter_context(tc.tile_pool(name="trp", bufs=2, space="PSUM"))
        st_psum = ctx.enter_context(tc.tile_pool(name="stp", bufs=2, space="PSUM"))
        proj_psum = ctx.enter_context(tc.tile_pool(name="pjp", bufs=3, space="PSUM"))
        sm_psum = ctx.enter_context(tc.tile_pool(name="smp", bufs=1, space="PSUM"))

        # identity first: the very first transposes need it
        ident_t = consts.tile([P, P], f32r, name="ident_t")
        nc.sync.dma_start(out=ident_t[:], in_=ident_in[:, :])
        ident = ident_t[:]

        # stage the first T-block's x rows before anything else so PE can
        # start transposing as early as possible
        xn_tiles = {}
        dmae = [nc.sync, nc.scalar]

        def load_xn(g):
            halves = []
            for h in range(2):
                t = xn_pool.tile([P, E // 2], f32r, tag="xn", name=f"xn_{g}_{h}")
                dmae[g % 2].dma_start(
                    out=t[:],
                    in_=xp[g * P : (g + 1) * P, h * (E // 2) : (h + 1) * (E // 2)],
                )
                halves.append(t)
            return halves

        # issue the low halves of the first block first: the e<4 transposes
        # depend only on them
        _pre = {tt: [None, None] for tt in range(4)}
        for h in range(2):
            for tt in range(4):
                t = xn_pool.tile([P, E // 2], f32r, tag="xn", name=f"xnp_{tt}_{h}")
                dmae[tt % 2].dma_start(
                    out=t[:],
                    in_=xp[tt * P : (tt + 1) * P, h * (E // 2) : (h + 1) * (E // 2)],
                )
                _pre[tt][h] = t
        for tt in range(4):
            xn_tiles[tt] = _pre[tt]

        ones = consts.tile([P, 1], f32r)
        nc.gpsimd.dma_start(out=ones[:], in_=ones_in[:, :])
        w_sb = {}
        for i, n in enumerate(("wk", "wv", "wq")):
            w_sb[n] = consts.tile([P, EC, D], f32r, name=f"w_{n}")
            dmae[i % 2].dma_start(
                out=w_sb[n][:], in_=w_in[n].rearrange("(c p) d -> p c d", p=P)
            )
        b_sb = {}
        for n in ("bq", "bk", "bv"):
            b_sb[n] = consts.tile([P, 1], f32, name=f"b_{n}")
            nc.gpsimd.dma_start(out=b_sb[n][:], in_=b_in[n][:, :])

        kt_tiles = []   # per-tb K^T [D, 512]
        v_tiles = []    # per-tb V natural [t_loc, 4, D]
        qt_tiles = {}   # per-qb Q^T [D, 512]
        tpos_sb = consts.tile([P, NT], f32)
        nc.gpsimd.dma_start(out=tpos_sb[:], in_=tpos_in[:, :])
        qpos_sb = consts.tile([P, 1024], f32)
        nc.sync.dma_start(
            out=qpos_sb[:],
            in_=bass.AP(
                tensor=qpos_in.ap().tensor,
                offset=0,
                ap=[[0, P], [1, 1024]],
            ),
        )

        # ---- phase 1: xT, projections ---------------------------------
        for tb in range(TB):
            xn = []
            for tt in range(4):
                g = tb * 4 + tt
                if g in xn_tiles:
                    t = xn_tiles[g]
                else:
                    t = load_xn(g)
                xn.append(t)

            xt = []
            for e in range(EC):
                tp = tr_psum.tile([P, 512], f32r, tag="tr")
                for tt in range(4):
                    half = xn[tt][e // 4]
                    nc.tensor.matmul(
                        tp[:, tt * P : (tt + 1) * P],
                        half[:, (e % 4) * P : (e % 4 + 1) * P],
                        ident,
                        is_transpose=True,
                        start=(tt == 0),
                        stop=(tt == 3),
                    )
                xte = xt_pool.tile([P, 512], f32r, tag="xte", name=f"xt_{tb}_{e}")
                if e % 2 == 0:
                    nc.vector.tensor_copy(xte[:], tp[:])
                else:
                    nc.scalar.copy(xte[:], tp[:])
                xt.append(xte)

            # K^T
            pp = proj_psum.tile([P, 512], f32, tag="pj")
            for e in range(EC):
                mm(pp[:], w_sb["wk"][:, e, :], xt[e][:], e == 0, e == EC - 1)
            kt = consts.tile([P, 512], f32r, name=f"kt_{tb}")
            nc.scalar.activation(
                out=kt[:], in_=pp[:], func=AF.Identity, bias=b_sb["bk"][:]
            )
            kt_tiles.append(kt)

            # V^T -> V natural
            pp = proj_psum.tile([P, 512], f32, tag="pj")
            for e in range(EC):
                mm(pp[:], w_sb["wv"][:, e, :], xt[e][:], e == 0, e == EC - 1)
            vt = sb_pool.tile([P, 512], f32r, tag="vt")
            nc.scalar.activation(
                out=vt[:], in_=pp[:], func=AF.Identity, bias=b_sb["bv"][:]
            )
            vp = tr_psum.tile([P, 512], f32r, tag="tr")
            for tt in range(4):
                nc.tensor.matmul(
                    vp[:, tt * P : (tt + 1) * P],
                    vt[:, tt * P : (tt + 1) * P],
                    ident,
                    is_transpose=True,
                    start=(tt == 0),
                    stop=(tt == 3),
                )
            v = consts.tile([P, 4, D], f32r, name=f"v_{tb}")
            nc.vector.tensor_copy(v[:], vp[:])
            v_tiles.append(v)

            # Q^T (q-block 0 lives at pos [0,512) = tb0; q-block 1 at tb2)
            if tb in (0, 2):
                qb = 0 if tb == 0 else 1
                pp = proj_psum.tile([P, 512], f32, tag="pj")
                for e in range(EC):
                    mm(pp[:], w_sb["wq"][:, e, :], xt[e][:], e == 0, e == EC - 1)
                qt = consts.tile([P, 512], f32r, name=f"qt_{qb}")
                nc.scalar.activation(
                    out=qt[:], in_=pp[:], func=AF.Identity, bias=b_sb["bq"][:]
                )
                qt_tiles[qb] = qt

        # ---- phase 2: attention ---------------------------------------
        ot_sb = out_pool.tile([P, 1024], f32)
        rs_sb = out_pool.tile([1, 1024], f32)
        for qb in (0, 1):
            n_t = QB_NT[qb]
            qt = qt_tiles[qb]
            qpos = qpos_sb[:, qb * 512 : (qb + 1) * 512]
            ot = proj_psum.tile([P, 512], f32, tag="pj")
            rs = sm_psum.tile([1, 512], f32, tag="sm")
            for j in range(n_t):
                st = st_psum.tile([P, 512], f32, tag="st")
                mm(st[:], kt_tiles[j // 4][:, (j % 4) * P : (j % 4 + 1) * P],
                   qt[:], True, True)
                pt = pt_pool.tile([P, 512], f32r, tag="pt")
                nc.scalar.activation(out=pt[:], in_=st[:], func=AF.Exp, scale=SCALE)
                nc.vector.scalar_tensor_tensor(
                    out=pt[:],
                    in0=qpos,
                    scalar=tpos_sb[:, j : j + 1],
                    in1=pt[:],
                    op0=mybir.AluOpType.is_ge,
                    op1=mybir.AluOpType.mult,
                )
                mm(ot[:], v_tiles[j // 4][:, j % 4, :], pt[:], j == 0, j == n_t - 1)
                mm(rs[:], ones[:], pt[:], j == 0, j == n_t - 1)

            nc.scalar.copy(rs_sb[0:1, qb * 512 : (qb + 1) * 512], rs[:])
            nc.vector.tensor_copy(ot_sb[:, qb * 512 : (qb + 1) * 512], ot[:])
            nc.sync.dma_start(
                out=ot_out[:, qb * 512 : (qb + 1) * 512],
                in_=ot_sb[:, qb * 512 : (qb + 1) * 512],
            )
            nc.scalar.dma_start(
                out=rs_out[:, qb * 512 : (qb + 1) * 512],
                in_=rs_sb[0:1, qb * 512 : (qb + 1) * 512],
            )

    nc.compile()
    return nc


_NC_CACHE = {}


def _get_nc():
    if "nc" not in _NC_CACHE:
        _NC_CACHE["nc"] = _build_nc()
    return _NC_CACHE["nc"]


def _get_runner():
    """Cached PJRT executable (same lowering as bass2jax.run_bass_via_pjrt,
    but the jitted function is built once and reused across calls)."""
    if "runner" in _NC_CACHE:
        return _NC_CACHE["runner"]

    import jax
    import jax.numpy as jnp
    from jax.sharding import Mesh, PartitionSpec
    from jax.experimental.shard_map import shard_map
    from concourse import bass2jax, mybir

    nc = _get_nc()
    bass2jax.install_neuronx_cc_hook()

    partition_name = nc.partition_id_tensor.name if nc.partition_id_tensor else None
    in_names, out_names, out_avals = [], [], []
    for alloc in nc.m.functions[0].allocations:
        if not isinstance(alloc, mybir.MemoryLocationSet):
            continue
        name = alloc.memorylocations[0].name
        if alloc.kind == "ExternalInput":
            if name != partition_name:
                in_names.append(name)
        elif alloc.kind == "ExternalOutput":
            out_names.append(name)
            out_avals.append(
                jax.core.ShapedArray(tuple(alloc.tensor_shape), mybir.dt.np(alloc.dtype))
            )
    n_params = len(in_names)
    n_outs = len(out_names)
    all_names = in_names + out_names
    if partition_name is not None:
        all_names = all_names + [partition_name]

    def _body(*args):
        operands = list(args)
        if partition_name is not None:
            operands.append(bass2jax.partition_id_tensor())
        outs = bass2jax._bass_exec_p.bind(
            *operands,
            out_avals=tuple(out_avals),
            in_names=tuple(all_names),
            out_names=tuple(out_names),
            lowering_input_output_aliases=(),
            sim_require_finite=True,
            sim_require_nnan=True,
            nc=nc,
        )
        return tuple(outs)

    devices = jax.devices()[:8]
    mesh = Mesh(__import__("numpy").asarray(devices), ("core",))
    sharded = jax.jit(
        shard_map(
            _body,
            mesh=mesh,
            in_specs=(PartitionSpec("core"),) * (n_params + n_outs),
            out_specs=(PartitionSpec("core"),) * n_outs,
            check_rep=False,
        ),
        donate_argnums=tuple(range(n_params, n_params + n_outs)),
        keep_unused=True,
    )
    runner = {
        "sharded": sharded,
        "in_names": in_names,
        "out_names": out_names,
        "out_avals": out_avals,
    }
    _NC_CACHE["runner"] = runner
    return runner


def _prep_in_concat(x, wq, bq, wk, bk, wv, bv):
    """Per-core in_maps, concatenated along axis 0 for shard_map."""
    x = np.asarray(x, dtype=np.float32)
    w = {
        "wq": np.asarray(wq, np.float32),
        "wk": np.asarray(wk, np.float32),
        "wv": np.asarray(wv, np.float32),
        "bq": np.asarray(bq, np.float32).reshape(P, 1),
        "bk": np.asarray(bk, np.float32).reshape(P, 1),
        "bv": np.asarray(bv, np.float32).reshape(P, 1),
    }
    if "perm" not in _NC_CACHE:
        _NC_CACHE["perm"] = [_role_perm(0), _role_perm(1)]
        tp, qp = [], []
        for role in (0, 1):
            perm = _NC_CACHE["perm"][role]
            tp.append(
                np.ascontiguousarray(
                    perm.reshape(NT, P).T.astype(np.float32)
                )
            )
            qp.append(
                np.ascontiguousarray(
                    np.concatenate([perm[0:512], perm[1024:1536]])
                    .astype(np.float32)
                    .reshape(1, 1024)
                )
            )
        _NC_CACHE["tpos"] = tp
        _NC_CACHE["qpos"] = qp
    perms = _NC_CACHE["perm"]

    runner = _get_runner()
    concat = {}
    concat["xp"] = np.concatenate(
        [x[c // 2][perms[c % 2]] for c in range(8)], axis=0
    )
    concat["tpos"] = np.concatenate([_NC_CACHE["tpos"][c % 2] for c in range(8)], axis=0)
    concat["ident"] = np.concatenate([np.eye(P, dtype=np.float32)] * 8, axis=0)
    concat["ones"] = np.ones((8 * P, 1), dtype=np.float32)
    concat["qpos"] = np.concatenate([_NC_CACHE["qpos"][c % 2] for c in range(8)], axis=0)
    for n, v in w.items():
        concat[n] = np.concatenate([v] * 8, axis=0)
    return [concat[n] for n in runner["in_names"]]


def _run_concat(concat_in):
    runner = _get_runner()
    zeros = [
        np.zeros((8 * a.shape[0], *a.shape[1:]), a.dtype) for a in runner["out_avals"]
    ]
    out_arrs = runner["sharded"](*concat_in, *zeros)
    ot = np.asarray(out_arrs[runner["out_names"].index("ot")]).reshape(8, P, 1024)
    rs = np.asarray(out_arrs[runner["out_names"].index("rs")]).reshape(8, 1024)
    return ot, rs


def _assemble(ot, rs):
    perms = _NC_CACHE["perm"]
    out = np.empty((B, S, D), dtype=np.float32)
    for c in range(8):
        b, role = divmod(c, 2)
        perm = perms[role]
        for qb, qpos0 in ((0, 0), (1, 1024)):
            otT = ot[c][:, qb * 512 : (qb + 1) * 512]          # [D, 512]
            rsq = rs[c][qb * 512 : (qb + 1) * 512]             # [512]
            out[b, perm[qpos0 : qpos0 + 512]] = (otT / rsq[None, :]).T
    return out


def kernel(x, wq, bq, wk, bk, wv, bv):
    concat_in = _prep_in_concat(x, wq, bq, wk, bk, wv, bv)
    ot, rs = _run_concat(concat_in)
    return _assemble(ot, rs)


def bench(x, wq, bq, wk, bk, wv, bv, iters=20):
    """Per-launch wall time with device-resident inputs (upper bound on HW exec)."""
    import time

    import jax

    runner = _get_runner()
    concat_in = _prep_in_concat(x, wq, bq, wk, bk, wv, bv)
    dev_in = [jax.device_put(a) for a in concat_in]
    for a in dev_in:
        a.block_until_ready()
    times = []
    for _ in range(iters):
        zeros = [
            np.zeros((8 * a.shape[0], *a.shape[1:]), a.dtype)
            for a in runner["out_avals"]
        ]
        t0 = time.perf_counter()
        out = runner["sharded"](*dev_in, *zeros)
        for a in out:
            a.block_until_ready()
        times.append(time.perf_counter() - t0)
    return times


def bench_chain(x, wq, bq, wk, bk, wv, bv, ks=(2, 12), reps=6):
    """Marginal device time per kernel launch: chain k sequential launches
    inside one jit (data-dependent via the mask input), compare wall."""
    import time

    import jax
    import jax.numpy as jnp
    from jax.sharding import Mesh, PartitionSpec
    from jax.experimental.shard_map import shard_map
    from concourse import bass2jax

    runner = _get_runner()
    nc = _get_nc()
    partition_name = nc.partition_id_tensor.name if nc.partition_id_tensor else None
    in_names = runner["in_names"]
    out_names = runner["out_names"]
    out_avals = runner["out_avals"]
    all_names = in_names + out_names + ([partition_name] if partition_name else [])
    mask_idx = in_names.index("mask")

    concat_in = _prep_in_concat(x, wq, bq, wk, bk, wv, bv)
    dev_in = [jax.device_put(a) for a in concat_in]
    for a in dev_in:
        a.block_until_ready()

    import numpy as _np

    def make_fn(k):
        def _body(*args):
            ins = list(args[: len(in_names)])
            zero_sets = args[len(in_names) :]
            o = None
            outs = None
            for i in range(k):
                cur = list(ins)
                if o is not None:
                    bump = (o[0:1, 0:1] != o[0:1, 0:1]).astype(jnp.uint8)
                    cur[mask_idx] = cur[mask_idx] | bump
                operands = cur + list(zero_sets[i * len(out_names) : (i + 1) * len(out_names)])
                if partition_name is not None:
                    operands.append(bass2jax.partition_id_tensor())
                outs = bass2jax._bass_exec_p.bind(
                    *operands,
                    out_avals=tuple(out_avals),
                    in_names=tuple(all_names),
                    out_names=tuple(out_names),
                    lowering_input_output_aliases=(),
                    sim_require_finite=True,
                    sim_require_nnan=True,
                    nc=nc,
                )
                o = outs[0]
            return tuple(outs)

        n_z = k * len(out_names)
        devices = jax.devices()[:8]
        mesh = Mesh(_np.asarray(devices), ("core",))
        return jax.jit(
            shard_map(
                _body,
                mesh=mesh,
                in_specs=(PartitionSpec("core"),) * (len(in_names) + n_z),
                out_specs=(PartitionSpec("core"),) * len(out_names),
                check_rep=False,
            ),
            donate_argnums=tuple(range(len(in_names), len(in_names) + n_z)),
            keep_unused=True,
        )

    results = {}
    for k in ks:
        fn = make_fn(k)
        walls = []
        for _ in range(reps):
            zeros = [
                _np.zeros((8 * a.shape[0], *a.shape[1:]), a.dtype)
                for _ in range(k)
                for a in out_avals
            ]
            t0 = time.perf_counter()
            out = fn(*dev_in, *zeros)
            for a in out:
                a.block_until_ready()
            walls.append(time.perf_counter() - t0)
        results[k] = min(walls)
    k0, k1 = ks
    per_launch = (results[k1] - results[k0]) / (k1 - k0)
    return per_launch, results


# revision 31
# speedup vs baseline: 1586.2113x; 1.0011x over previous
"""Causal single-head attention (B=4, S=2048, E=1024, D=128) on 8 trn2 cores.

Sharding: 2 cores per batch. Each core computes the attention output for
1024 query rows of its batch. To keep one uniform SPMD program while
balancing the causal (triangular) work, the host permutes each batch's
rows per core role and ships a per-core 0/1 mask table:

  role 0: perm = [0:512 | 512:1024 | 1536:2048 | 1024:1536]
  role 1: perm = [512:1024 | 0:512 | 1024:1536 | 1536:2048]

Queries are the permuted positions [0,512) (q-block 0, key extent 1024)
and [1024,1536) (q-block 1, key extent 2048). Both roles then run the
exact same static program; causality (including wasted padded tiles) is
enforced by multiplying exp(scores) with the host-baked mask.

Per-core kernel (flow over transposed scores, fp32r matmuls):
  xT = PE-transpose of x (E on partitions)
  K^T/V^T/Q^T = w.T @ xT accumulated over 8 E-chunks; V re-transposed
  per q-block, per key tile j: st[t,s] = KT_j.T @ QT ; pt = exp(st*scale)
  pt *= mask ; rowacc += pt ; outT[D,s] += V_j.T @ pt
  rowsum via ones-matmul per 128-col chunk; out = transpose(outT) * 1/rowsum
"""

import math

import numpy as np

B, S, E, D = 4, 2048, 1024, 128
P = 128
EC = E // P          # 8 E-chunks
NT = S // P          # 16 key tiles
TB = S // 512        # 4 key blocks of 512
QB_NT = (8, 16)      # key-tile extent per q-block (padded, role-uniform)
N_MASK = QB_NT[0] + QB_NT[1]
SCALE = 1.0 / math.sqrt(D)

MM_DT = None  # set in _build_nc (float32r)


def _role_perm(role):
    a = np.arange
    if role == 0:
        blocks = [a(0, 512), a(512, 1024), a(1536, 2048), a(1024, 1536)]
    else:
        blocks = [a(512, 1024), a(0, 512), a(1024, 1536), a(1536, 2048)]
    return np.concatenate(blocks)


def _role_mask(role):
    """[128, N_MASK*512] u8: concat over (qb, j) of valid(t_pos, s_pos)."""
    perm = _role_perm(role)
    tiles = []
    for qb, qpos0 in ((0, 0), (1, 1024)):
        q_orig = perm[qpos0 : qpos0 + 512]
        for j in range(QB_NT[qb]):
            t_orig = perm[j * P : (j + 1) * P]
            tiles.append((t_orig[:, None] <= q_orig[None, :]).astype(np.uint8))
    return np.concatenate(tiles, axis=1)


def _build_nc():
    global MM_DT
    from contextlib import ExitStack

    import concourse.bass as bass
    import concourse.tile as tile
    from concourse import bacc, masks, mybir

    MM_DT = mybir.dt.float32r
    f32r = mybir.dt.float32r
    f32 = mybir.dt.float32
    u8 = mybir.dt.uint8
    AF = mybir.ActivationFunctionType

    nc = bacc.Bacc("TRN2", target_bir_lowering=False, debug=False)

    xp = nc.dram_tensor("xp", [S, E], f32r, kind="ExternalInput")
    w_in = {
        n: nc.dram_tensor(n, [E, D], f32r, kind="ExternalInput")
        for n in ("wq", "wk", "wv")
    }
    b_in = {
        n: nc.dram_tensor(n, [P, 1], f32, kind="ExternalInput")
        for n in ("bq", "bk", "bv")
    }
    tpos_in = nc.dram_tensor("tpos", [P, NT], f32, kind="ExternalInput")
    ident_in = nc.dram_tensor("ident", [P, P], f32r, kind="ExternalInput")
    ones_in = nc.dram_tensor("ones", [P, 1], f32r, kind="ExternalInput")
    onesr_in = nc.dram_tensor("onesr", [1, P], f32r, kind="ExternalInput")
    qpos_in = nc.dram_tensor("qpos", [1, 1024], f32r, kind="ExternalInput")
    ot_out = nc.dram_tensor("ot", [P, 1024], f32, kind="ExternalOutput")
    rs_out = nc.dram_tensor("rs", [1, 1024], f32, kind="ExternalOutput")

    def mm(out, lhsT, rhs, start, stop):
        nc.tensor.matmul(out, lhsT, rhs, start=start, stop=stop)

    with tile.TileContext(nc) as tc, ExitStack() as ctx:
        consts = ctx.enter_context(tc.tile_pool(name="consts", bufs=1))
        xn_pool = ctx.enter_context(tc.tile_pool(name="xn", bufs=16))
        xt_pool = ctx.enter_context(tc.tile_pool(name="xt", bufs=16))
        sb_pool = ctx.enter_context(tc.tile_pool(name="sb", bufs=2))
        pt_pool = ctx.enter_context(tc.tile_pool(name="pt", bufs=6))
        out_pool = ctx.enter_context(tc.tile_pool(name="outp", bufs=1))
        tr_psum = ctx.enter_context(tc.tile_pool(name="trp", bufs=2, space="PSUM"))
        st_psum = ctx.enter_context(tc.tile_pool(name="stp", bufs=2, space="PSUM"))
        proj_psum = ctx.enter_context(tc.tile_pool(name="pjp", bufs=3, space="PSUM"))
        sm_psum = ctx.enter_context(tc.tile_pool(name="smp", bufs=1, space="PSUM"))

        # identity first: the very first transposes need it
        ident_t = consts.tile([P, P], f32r, name="ident_t")
        nc.sync.dma_start(out=ident_t[:], in_=ident_in[:, :])
        ident = ident_t[:]

        # stage the first T-block's x rows before anything else so PE can
        # start transposing as early as possible
        xn_tiles = {}
        dmae = [nc.sync, nc.scalar]

        def load_xn(g):
            halves = []
            for h in range(2):
                t = xn_pool.tile([P, E // 2], f32r, tag="xn", name=f"xn_{g}_{h}")
                dmae[g % 2].dma_start(
                    out=t[:],
                    in_=xp[g * P : (g + 1) * P, h * (E // 2) : (h + 1) * (E // 2)],
                )
                halves.append(t)
            return halves

        # issue the low halves of the first block first: the e<4 transposes
        # depend only on them
        _pre = {tt: [None, None] for tt in range(4)}
        for h in range(2):
            for tt in range(4):
                t = xn_pool.tile([P, E // 2], f32r, tag="xn", name=f"xnp_{tt}_{h}")
                dmae[tt % 2].dma_start(
                    out=t[:],
                    in_=xp[tt * P : (tt + 1) * P, h * (E // 2) : (h + 1) * (E // 2)],
                )
                _pre[tt][h] = t
        for tt in range(4):
            xn_tiles[tt] = _pre[tt]

        ones = consts.tile([P, 1], f32r)
        nc.gpsimd.dma_start(out=ones[:], in_=ones_in[:, :])
        onesr = consts.tile([1, P], f32r)
        nc.sync.dma_start(out=onesr[:], in_=onesr_in[:, :])
        w_sb = {}
        for i, n in enumerate(("wk", "wv", "wq")):
            w_sb[n] = consts.tile([P, EC, D], f32r, name=f"w_{n}")
            dmae[i % 2].dma_start(
                out=w_sb[n][:], in_=w_in[n].rearrange("(c p) d -> p c d", p=P)
            )
        b_sb = {}
        for n in ("bq", "bk", "bv"):
            b_sb[n] = consts.tile([P, 1], f32, name=f"b_{n}")
            nc.gpsimd.dma_start(out=b_sb[n][:], in_=b_in[n][:, :])

        kt_tiles = []   # per-tb K^T [D, 512]
        v_tiles = []    # per-tb V natural [t_loc, 4, D]
        qt_tiles = {}   # per-qb Q^T [D, 512]
        tpos_sb = consts.tile([P, NT], f32)
        nc.gpsimd.dma_start(out=tpos_sb[:], in_=tpos_in[:, :])
        qpos1 = consts.tile([1, 1024], f32r)
        nc.sync.dma_start(out=qpos1[:], in_=qpos_in[:, :])
        qpos_sb = consts.tile([P, 1024], f32)
        for h in range(2):
            qb_ps = sm_psum.tile([P, 512], f32, tag="sm", name=f"qbps_{h}")
            nc.tensor.matmul(
                qb_ps[:], onesr[:], qpos1[0:1, h * 512 : (h + 1) * 512],
                start=True, stop=True,
            )
            nc.vector.tensor_copy(qpos_sb[:, h * 512 : (h + 1) * 512], qb_ps[:])

        # ---- phase 1: xT, projections ---------------------------------
        for tb in range(TB):
            xn = []
            for tt in range(4):
                g = tb * 4 + tt
                if g in xn_tiles:
                    t = xn_tiles[g]
                else:
                    t = load_xn(g)
                xn.append(t)

            xt = []
            for e in range(EC):
                tp = tr_psum.tile([P, 512], f32r, tag="tr")
                for tt in range(4):
                    half = xn[tt][e // 4]
                    nc.tensor.matmul(
                        tp[:, tt * P : (tt + 1) * P],
                        half[:, (e % 4) * P : (e % 4 + 1) * P],
                        ident,
                        is_transpose=True,
                        start=(tt == 0),
                        stop=(tt == 3),
                    )
                xte = xt_pool.tile([P, 512], f32r, tag="xte", name=f"xt_{tb}_{e}")
                if e % 2 == 0:
                    nc.vector.tensor_copy(xte[:], tp[:])
                else:
                    nc.scalar.copy(xte[:], tp[:])
                xt.append(xte)

            # K^T
            pp = proj_psum.tile([P, 512], f32, tag="pj")
            for e in range(EC):
                mm(pp[:], w_sb["wk"][:, e, :], xt[e][:], e == 0, e == EC - 1)
            kt = consts.tile([P, 512], f32r, name=f"kt_{tb}")
            nc.scalar.activation(
                out=kt[:], in_=pp[:], func=AF.Identity, bias=b_sb["bk"][:]
            )
            kt_tiles.append(kt)

            # V^T -> V natural
            pp = proj_psum.tile([P, 512], f32, tag="pj")
            for e in range(EC):
                mm(pp[:], w_sb["wv"][:, e, :], xt[e][:], e == 0, e == EC - 1)
            vt = sb_pool.tile([P, 512], f32r, tag="vt")
            nc.scalar.activation(
                out=vt[:], in_=pp[:], func=AF.Identity, bias=b_sb["bv"][:]
            )
            vp = tr_psum.tile([P, 512], f32r, tag="tr")
            for tt in range(4):
                nc.tensor.matmul(
                    vp[:, tt * P : (tt + 1) * P],
                    vt[:, tt * P : (tt + 1) * P],
                    ident,
                    is_transpose=True,
                    start=(tt == 0),
                    stop=(tt == 3),
                )
            v = consts.tile([P, 4, D], f32r, name=f"v_{tb}")
            nc.vector.tensor_copy(v[:], vp[:])
            v_tiles.append(v)

            # Q^T (q-block 0 lives at pos [0,512) = tb0; q-block 1 at tb2)
            if tb in (0, 2):
                qb = 0 if tb == 0 else 1
                pp = proj_psum.tile([P, 512], f32, tag="pj")
                for e in range(EC):
                    mm(pp[:], w_sb["wq"][:, e, :], xt[e][:], e == 0, e == EC - 1)
                qt = consts.tile([P, 512], f32r, name=f"qt_{qb}")
                nc.scalar.activation(
                    out=qt[:], in_=pp[:], func=AF.Identity, bias=b_sb["bq"][:]
                )
                qt_tiles[qb] = qt

        # ---- phase 2: attention ---------------------------------------
        ot_sb = out_pool.tile([P, 1024], f32)
        rs_sb = out_pool.tile([1, 1024], f32)
        for qb in (0, 1):
            n_t = QB_NT[qb]
            qt = qt_tiles[qb]
            qpos = qpos_sb[:, qb * 512 : (qb + 1) * 512]
            ot = proj_psum.tile([P, 512], f32, tag="pj")
            rs = sm_psum.tile([1, 512], f32, tag="sm")
            for j in range(n_t):
                st = st_psum.tile([P, 512], f32, tag="st")
                mm(st[:], kt_tiles[j // 4][:, (j % 4) * P : (j % 4 + 1) * P],
                   qt[:], True, True)
                pt = pt_pool.tile([P, 512], f32r, tag="pt")
                nc.scalar.activation(out=pt[:], in_=st[:], func=AF.Exp, scale=SCALE)
                nc.vector.scalar_tensor_tensor(
                    out=pt[:],
                    in0=qpos,
                    scalar=tpos_sb[:, j : j + 1],
                    in1=pt[:],
                    op0=mybir.AluOpType.is_ge,
                    op1=mybir.AluOpType.mult,
                )
                mm(ot[:], v_tiles[j // 4][:, j % 4, :], pt[:], j == 0, j == n_t - 1)
                mm(rs[:], ones[:], pt[:], j == 0, j == n_t - 1)

            nc.scalar.copy(rs_sb[0:1, qb * 512 : (qb + 1) * 512], rs[:])
            nc.vector.tensor_copy(ot_sb[:, qb * 512 : (qb + 1) * 512], ot[:])
            nc.sync.dma_start(
                out=ot_out[:, qb * 512 : (qb + 1) * 512],
                in_=ot_sb[:, qb * 512 : (qb + 1) * 512],
            )
            nc.scalar.dma_start(
                out=rs_out[:, qb * 512 : (qb + 1) * 512],
                in_=rs_sb[0:1, qb * 512 : (qb + 1) * 512],
            )

    nc.compile()
    return nc


_NC_CACHE = {}


def _get_nc():
    if "nc" not in _NC_CACHE:
        _NC_CACHE["nc"] = _build_nc()
    return _NC_CACHE["nc"]


def _get_runner():
    """Cached PJRT executable (same lowering as bass2jax.run_bass_via_pjrt,
    but the jitted function is built once and reused across calls)."""
    if "runner" in _NC_CACHE:
        return _NC_CACHE["runner"]

    import jax
    import jax.numpy as jnp
    from jax.sharding import Mesh, PartitionSpec
    from jax.experimental.shard_map import shard_map
    from concourse import bass2jax, mybir

    nc = _get_nc()
    bass2jax.install_neuronx_cc_hook()

    partition_name = nc.partition_id_tensor.name if nc.partition_id_tensor else None
    in_names, out_names, out_avals = [], [], []
    for alloc in nc.m.functions[0].allocations:
        if not isinstance(alloc, mybir.MemoryLocationSet):
            continue
        name = alloc.memorylocations[0].name
        if alloc.kind == "ExternalInput":
            if name != partition_name:
                in_names.append(name)
        elif alloc.kind == "ExternalOutput":
            out_names.append(name)
            out_avals.append(
                jax.core.ShapedArray(tuple(alloc.tensor_shape), mybir.dt.np(alloc.dtype))
            )
    n_params = len(in_names)
    n_outs = len(out_names)
    all_names = in_names + out_names
    if partition_name is not None:
        all_names = all_names + [partition_name]

    def _body(*args):
        operands = list(args)
        if partition_name is not None:
            operands.append(bass2jax.partition_id_tensor())
        outs = bass2jax._bass_exec_p.bind(
            *operands,
            out_avals=tuple(out_avals),
            in_names=tuple(all_names),
            out_names=tuple(out_names),
            lowering_input_output_aliases=(),
            sim_require_finite=True,
            sim_require_nnan=True,
            nc=nc,
        )
        return tuple(outs)

    devices = jax.devices()[:8]
    mesh = Mesh(__import__("numpy").asarray(devices), ("core",))
    sharded = jax.jit(
        shard_map(
            _body,
            mesh=mesh,
            in_specs=(PartitionSpec("core"),) * (n_params + n_outs),
            out_specs=(PartitionSpec("core"),) * n_outs,
            check_rep=False,
        ),
        donate_argnums=tuple(range(n_params, n_params + n_outs)),
        keep_unused=True,
    )
    runner = {
        "sharded": sharded,
        "in_names": in_names,
        "out_names": out_names,
        "out_avals": out_avals,
    }
    _NC_CACHE["runner"] = runner
    return runner


def _prep_in_concat(x, wq, bq, wk, bk, wv, bv):
    """Per-core in_maps, concatenated along axis 0 for shard_map."""
    x = np.asarray(x, dtype=np.float32)
    w = {
        "wq": np.asarray(wq, np.float32),
        "wk": np.asarray(wk, np.float32),
        "wv": np.asarray(wv, np.float32),
        "bq": np.asarray(bq, np.float32).reshape(P, 1),
        "bk": np.asarray(bk, np.float32).reshape(P, 1),
        "bv": np.asarray(bv, np.float32).reshape(P, 1),
    }
    if "perm" not in _NC_CACHE:
        _NC_CACHE["perm"] = [_role_perm(0), _role_perm(1)]
        tp, qp = [], []
        for role in (0, 1):
            perm = _NC_CACHE["perm"][role]
            tp.append(
                np.ascontiguousarray(
                    perm.reshape(NT, P).T.astype(np.float32)
                )
            )
            qp.append(
                np.ascontiguousarray(
                    np.concatenate([perm[0:512], perm[1024:1536]])
                    .astype(np.float32)
                    .reshape(1, 1024)
                )
            )
        _NC_CACHE["tpos"] = tp
        _NC_CACHE["qpos"] = qp
    perms = _NC_CACHE["perm"]

    runner = _get_runner()
    concat = {}
    concat["xp"] = np.concatenate(
        [x[c // 2][perms[c % 2]] for c in range(8)], axis=0
    )
    concat["tpos"] = np.concatenate([_NC_CACHE["tpos"][c % 2] for c in range(8)], axis=0)
    concat["ident"] = np.concatenate([np.eye(P, dtype=np.float32)] * 8, axis=0)
    concat["ones"] = np.ones((8 * P, 1), dtype=np.float32)
    concat["onesr"] = np.ones((8, P), dtype=np.float32)
    concat["qpos"] = np.concatenate([_NC_CACHE["qpos"][c % 2] for c in range(8)], axis=0)
    for n, v in w.items():
        concat[n] = np.concatenate([v] * 8, axis=0)
    return [concat[n] for n in runner["in_names"]]


def _run_concat(concat_in):
    runner = _get_runner()
    zeros = [
        np.zeros((8 * a.shape[0], *a.shape[1:]), a.dtype) for a in runner["out_avals"]
    ]
    out_arrs = runner["sharded"](*concat_in, *zeros)
    ot = np.asarray(out_arrs[runner["out_names"].index("ot")]).reshape(8, P, 1024)
    rs = np.asarray(out_arrs[runner["out_names"].index("rs")]).reshape(8, 1024)
    return ot, rs


def _assemble(ot, rs):
    perms = _NC_CACHE["perm"]
    out = np.empty((B, S, D), dtype=np.float32)
    for c in range(8):
        b, role = divmod(c, 2)
        perm = perms[role]
        for qb, qpos0 in ((0, 0), (1, 1024)):
            otT = ot[c][:, qb * 512 : (qb + 1) * 512]          # [D, 512]
            rsq = rs[c][qb * 512 : (qb + 1) * 512]             # [512]
            out[b, perm[qpos0 : qpos0 + 512]] = (otT / rsq[None, :]).T
    return out


def kernel(x, wq, bq, wk, bk, wv, bv):
    concat_in = _prep_in_concat(x, wq, bq, wk, bk, wv, bv)
    ot, rs = _run_concat(concat_in)
    return _assemble(ot, rs)


def bench(x, wq, bq, wk, bk, wv, bv, iters=20):
    """Per-launch wall time with device-resident inputs (upper bound on HW exec)."""
    import time

    import jax

    runner = _get_runner()
    concat_in = _prep_in_concat(x, wq, bq, wk, bk, wv, bv)
    dev_in = [jax.device_put(a) for a in concat_in]
    for a in dev_in:
        a.block_until_ready()
    times = []
    for _ in range(iters):
        zeros = [
            np.zeros((8 * a.shape[0], *a.shape[1:]), a.dtype)
            for a in runner["out_avals"]
        ]
        t0 = time.perf_counter()
        out = runner["sharded"](*dev_in, *zeros)
        for a in out:
            a.block_until_ready()
        times.append(time.perf_counter() - t0)
    return times


def bench_chain(x, wq, bq, wk, bk, wv, bv, ks=(2, 12), reps=6):
    """Marginal device time per kernel launch: chain k sequential launches
    inside one jit (data-dependent via the mask input), compare wall."""
    import time

    import jax
    import jax.numpy as jnp
    from jax.sharding import Mesh, PartitionSpec
    from jax.experimental.shard_map import shard_map
    from concourse import bass2jax

    runner = _get_runner()
    nc = _get_nc()
    partition_name = nc.partition_id_tensor.name if nc.partition_id_tensor else None
    in_names = runner["in_names"]
    out_names = runner["out_names"]
    out_avals = runner["out_avals"]
    all_names = in_names + out_names + ([partition_name] if partition_name else [])
    mask_idx = in_names.index("mask")

    concat_in = _prep_in_concat(x, wq, bq, wk, bk, wv, bv)
    dev_in = [jax.device_put(a) for a in concat_in]
    for a in dev_in:
        a.block_until_ready()

    import numpy as _np

    def make_fn(k):
        def _body(*args):
            ins = list(args[: len(in_names)])
            zero_sets = args[len(in_names) :]
            o = None
            outs = None
            for i in range(k):
                cur = list(ins)
                if o is not None:
                    bump = (o[0:1, 0:1] != o[0:1, 0:1]).astype(jnp.uint8)
                    cur[mask_idx] = cur[mask_idx] | bump
                operands = cur + list(zero_sets[i * len(out_names) : (i + 1) * len(out_names)])
                if partition_name is not None:
                    operands.append(bass2jax.partition_id_tensor())
                outs = bass2jax._bass_exec_p.bind(
                    *operands,
                    out_avals=tuple(out_avals),
                    in_names=tuple(all_names),
                    out_names=tuple(out_names),
                    lowering_input_output_aliases=(),
                    sim_require_finite=True,
                    sim_require_nnan=True,
                    nc=nc,
                )
                o = outs[0]
            return tuple(outs)

        n_z = k * len(out_names)
        devices = jax.devices()[:8]
        mesh = Mesh(_np.asarray(devices), ("core",))
        return jax.jit(
            shard_map(
                _body,
                mesh=mesh,
                in_specs=(PartitionSpec("core"),) * (len(in_names) + n_z),
                out_specs=(PartitionSpec("core"),) * len(out_names),
                check_rep=False,
            ),
            donate_argnums=tuple(range(len(in_names), len(in_names) + n_z)),
            keep_unused=True,
        )

    results = {}
    for k in ks:
        fn = make_fn(k)
        walls = []
        for _ in range(reps):
            zeros = [
                _np.zeros((8 * a.shape[0], *a.shape[1:]), a.dtype)
                for _ in range(k)
                for a in out_avals
            ]
            t0 = time.perf_counter()
            out = fn(*dev_in, *zeros)
            for a in out:
                a.block_until_ready()
            walls.append(time.perf_counter() - t0)
        results[k] = min(walls)
    k0, k1 = ks
    per_launch = (results[k1] - results[k0]) / (k1 - k0)
    return per_launch, results


# revision 35
# speedup vs baseline: 1599.3219x; 1.0083x over previous
"""Causal single-head attention (B=4, S=2048, E=1024, D=128) on 8 trn2 cores.

Sharding: 2 cores per batch. Each core computes the attention output for
1024 query rows of its batch. To keep one uniform SPMD program while
balancing the causal (triangular) work, the host permutes each batch's
rows per core role and ships a per-core 0/1 mask table:

  role 0: perm = [0:512 | 512:1024 | 1536:2048 | 1024:1536]
  role 1: perm = [512:1024 | 0:512 | 1024:1536 | 1536:2048]

Queries are the permuted positions [0,512) (q-block 0, key extent 1024)
and [1024,1536) (q-block 1, key extent 2048). Both roles then run the
exact same static program; causality (including wasted padded tiles) is
enforced by multiplying exp(scores) with the host-baked mask.

Per-core kernel (flow over transposed scores, fp32r matmuls):
  xT = PE-transpose of x (E on partitions)
  K^T/V^T/Q^T = w.T @ xT accumulated over 8 E-chunks; V re-transposed
  per q-block, per key tile j: st[t,s] = KT_j.T @ QT ; pt = exp(st*scale)
  pt *= mask ; rowacc += pt ; outT[D,s] += V_j.T @ pt
  rowsum via ones-matmul per 128-col chunk; out = transpose(outT) * 1/rowsum
"""

import math

import numpy as np

B, S, E, D = 4, 2048, 1024, 128
P = 128
EC = E // P          # 8 E-chunks
NT = S // P          # 16 key tiles
TB = S // 512        # 4 key blocks of 512
QB_NT = (8, 16)      # key-tile extent per q-block (padded, role-uniform)
N_MASK = QB_NT[0] + QB_NT[1]
SCALE = 1.0 / math.sqrt(D)

MM_DT = None  # set in _build_nc (float32r)


def _role_perm(role):
    a = np.arange
    if role == 0:
        blocks = [a(0, 512), a(512, 1024), a(1536, 2048), a(1024, 1536)]
    else:
        blocks = [a(512, 1024), a(0, 512), a(1024, 1536), a(1536, 2048)]
    return np.concatenate(blocks)


def _role_mask(role):
    """[128, N_MASK*512] u8: concat over (qb, j) of valid(t_pos, s_pos)."""
    perm = _role_perm(role)
    tiles = []
    for qb, qpos0 in ((0, 0), (1, 1024)):
        q_orig = perm[qpos0 : qpos0 + 512]
        for j in range(QB_NT[qb]):
            t_orig = perm[j * P : (j + 1) * P]
            tiles.append((t_orig[:, None] <= q_orig[None, :]).astype(np.uint8))
    return np.concatenate(tiles, axis=1)


def _build_nc():
    global MM_DT
    from contextlib import ExitStack

    import concourse.bass as bass
    import concourse.tile as tile
    from concourse import bacc, masks, mybir

    MM_DT = mybir.dt.float32r
    f32r = mybir.dt.float32r
    f32 = mybir.dt.float32
    u8 = mybir.dt.uint8
    AF = mybir.ActivationFunctionType

    nc = bacc.Bacc("TRN2", target_bir_lowering=False, debug=False)

    xp = nc.dram_tensor("xp", [S, E], f32r, kind="ExternalInput")
    w_in = {
        n: nc.dram_tensor(n, [E, D], f32r, kind="ExternalInput")
        for n in ("wq", "wk", "wv")
    }
    b_in = {
        n: nc.dram_tensor(n, [P, 1], f32, kind="ExternalInput")
        for n in ("bq", "bk", "bv")
    }
    tpos_in = nc.dram_tensor("tpos", [P, NT], f32, kind="ExternalInput")
    ident_in = nc.dram_tensor("ident", [P, P], f32r, kind="ExternalInput")
    ones_in = nc.dram_tensor("ones", [P, 1], f32r, kind="ExternalInput")
    onesr_in = nc.dram_tensor("onesr", [1, P], f32r, kind="ExternalInput")
    qpos_in = nc.dram_tensor("qpos", [1, 1024], f32r, kind="ExternalInput")
    ot_out = nc.dram_tensor("ot", [P, 1024], f32, kind="ExternalOutput")
    rs_out = nc.dram_tensor("rs", [1, 1024], f32, kind="ExternalOutput")

    def mm(out, lhsT, rhs, start, stop):
        nc.tensor.matmul(out, lhsT, rhs, start=start, stop=stop)

    with tile.TileContext(nc) as tc, ExitStack() as ctx:
        consts = ctx.enter_context(tc.tile_pool(name="consts", bufs=1))
        xn_pool = ctx.enter_context(tc.tile_pool(name="xn", bufs=32))
        xt_pool = ctx.enter_context(tc.tile_pool(name="xt", bufs=24))
        sb_pool = ctx.enter_context(tc.tile_pool(name="sb", bufs=2))
        pt_pool = ctx.enter_context(tc.tile_pool(name="pt", bufs=8))
        out_pool = ctx.enter_context(tc.tile_pool(name="outp", bufs=1))
        tr_psum = ctx.enter_context(tc.tile_pool(name="trp", bufs=2, space="PSUM"))
        st_psum = ctx.enter_context(tc.tile_pool(name="stp", bufs=2, space="PSUM"))
        proj_psum = ctx.enter_context(tc.tile_pool(name="pjp", bufs=3, space="PSUM"))
        sm_psum = ctx.enter_context(tc.tile_pool(name="smp", bufs=1, space="PSUM"))

        # identity first: the very first transposes need it
        ident_t = consts.tile([P, P], f32r, name="ident_t")
        nc.sync.dma_start(out=ident_t[:], in_=ident_in[:, :])
        ident = ident_t[:]

        # stage the first T-block's x rows before anything else so PE can
        # start transposing as early as possible
        xn_tiles = {}
        dmae = [nc.sync, nc.scalar]

        def load_xn(g):
            halves = []
            for h in range(2):
                t = xn_pool.tile([P, E // 2], f32r, tag="xn", name=f"xn_{g}_{h}")
                dmae[g % 2].dma_start(
                    out=t[:],
                    in_=xp[g * P : (g + 1) * P, h * (E // 2) : (h + 1) * (E // 2)],
                )
                halves.append(t)
            return halves

        # issue the low halves of the first block first: the e<4 transposes
        # depend only on them
        _pre = {tt: [None, None] for tt in range(4)}
        for h in range(2):
            for tt in range(4):
                t = xn_pool.tile([P, E // 2], f32r, tag="xn", name=f"xnp_{tt}_{h}")
                dmae[tt % 2].dma_start(
                    out=t[:],
                    in_=xp[tt * P : (tt + 1) * P, h * (E // 2) : (h + 1) * (E // 2)],
                )
                _pre[tt][h] = t
        for tt in range(4):
            xn_tiles[tt] = _pre[tt]

        ones = consts.tile([P, 1], f32r)
        nc.gpsimd.dma_start(out=ones[:], in_=ones_in[:, :])
        onesr = consts.tile([1, P], f32r)
        nc.sync.dma_start(out=onesr[:], in_=onesr_in[:, :])
        w_sb = {}
        for i, n in enumerate(("wk", "wv", "wq")):
            w_sb[n] = consts.tile([P, EC, D], f32r, name=f"w_{n}")
            dmae[i % 2].dma_start(
                out=w_sb[n][:], in_=w_in[n].rearrange("(c p) d -> p c d", p=P)
            )
        b_sb = {}
        for n in ("bq", "bk", "bv"):
            b_sb[n] = consts.tile([P, 1], f32, name=f"b_{n}")
            nc.gpsimd.dma_start(out=b_sb[n][:], in_=b_in[n][:, :])

        kt_tiles = []   # per-tb K^T [D, 512]
        v_tiles = []    # per-tb V natural [t_loc, 4, D]
        qt_tiles = {}   # per-qb Q^T [D, 512]
        tpos_sb = consts.tile([P, NT], f32)
        nc.gpsimd.dma_start(out=tpos_sb[:], in_=tpos_in[:, :])
        qpos1 = consts.tile([1, 1024], f32r)
        nc.sync.dma_start(out=qpos1[:], in_=qpos_in[:, :])
        qpos_sb = consts.tile([P, 1024], f32)
        for h in range(2):
            qb_ps = sm_psum.tile([P, 512], f32, tag="sm", name=f"qbps_{h}")
            nc.tensor.matmul(
                qb_ps[:], onesr[:], qpos1[0:1, h * 512 : (h + 1) * 512],
                start=True, stop=True,
            )
            nc.vector.tensor_copy(qpos_sb[:, h * 512 : (h + 1) * 512], qb_ps[:])

        # ---- phase 1: xT, projections ---------------------------------
        for tb in range(TB):
            xn = []
            for tt in range(4):
                g = tb * 4 + tt
                if g in xn_tiles:
                    t = xn_tiles[g]
                else:
                    t = load_xn(g)
                xn.append(t)

            xt = []
            for e in range(EC):
                tp = tr_psum.tile([P, 512], f32r, tag="tr")
                for tt in range(4):
                    half = xn[tt][e // 4]
                    nc.tensor.matmul(
                        tp[:, tt * P : (tt + 1) * P],
                        half[:, (e % 4) * P : (e % 4 + 1) * P],
                        ident,
                        is_transpose=True,
                        start=(tt == 0),
                        stop=(tt == 3),
                    )
                xte = xt_pool.tile([P, 512], f32r, tag="xte", name=f"xt_{tb}_{e}")
                if e % 2 == 0:
                    nc.vector.tensor_copy(xte[:], tp[:])
                else:
                    nc.scalar.copy(xte[:], tp[:])
                xt.append(xte)

            # K^T
            pp = proj_psum.tile([P, 512], f32, tag="pj")
            for e in range(EC):
                mm(pp[:], w_sb["wk"][:, e, :], xt[e][:], e == 0, e == EC - 1)
            kt = consts.tile([P, 512], f32r, name=f"kt_{tb}")
            nc.scalar.activation(
                out=kt[:], in_=pp[:], func=AF.Identity, bias=b_sb["bk"][:]
            )
            kt_tiles.append(kt)

            # V^T -> V natural
            pp = proj_psum.tile([P, 512], f32, tag="pj")
            for e in range(EC):
                mm(pp[:], w_sb["wv"][:, e, :], xt[e][:], e == 0, e == EC - 1)
            vt = sb_pool.tile([P, 512], f32r, tag="vt")
            nc.scalar.activation(
                out=vt[:], in_=pp[:], func=AF.Identity, bias=b_sb["bv"][:]
            )
            vp = tr_psum.tile([P, 512], f32r, tag="tr")
            for tt in range(4):
                nc.tensor.matmul(
                    vp[:, tt * P : (tt + 1) * P],
                    vt[:, tt * P : (tt + 1) * P],
                    ident,
                    is_transpose=True,
                    start=(tt == 0),
                    stop=(tt == 3),
                )
            v = consts.tile([P, 4, D], f32r, name=f"v_{tb}")
            nc.vector.tensor_copy(v[:], vp[:])
            v_tiles.append(v)

            # Q^T (q-block 0 lives at pos [0,512) = tb0; q-block 1 at tb2)
            if tb in (0, 2):
                qb = 0 if tb == 0 else 1
                pp = proj_psum.tile([P, 512], f32, tag="pj")
                for e in range(EC):
                    mm(pp[:], w_sb["wq"][:, e, :], xt[e][:], e == 0, e == EC - 1)
                qt = consts.tile([P, 512], f32r, name=f"qt_{qb}")
                nc.scalar.activation(
                    out=qt[:], in_=pp[:], func=AF.Identity, bias=b_sb["bq"][:]
                )
                qt_tiles[qb] = qt

        # ---- phase 2: attention ---------------------------------------
        ot_sb = out_pool.tile([P, 1024], f32)
        rs_sb = out_pool.tile([1, 1024], f32)
        for qb in (0, 1):
            n_t = QB_NT[qb]
            qt = qt_tiles[qb]
            qpos = qpos_sb[:, qb * 512 : (qb + 1) * 512]
            ot = proj_psum.tile([P, 512], f32, tag="pj")
            rs = sm_psum.tile([1, 512], f32, tag="sm")
            for j in range(n_t):
                st = st_psum.tile([P, 512], f32, tag="st")
                mm(st[:], kt_tiles[j // 4][:, (j % 4) * P : (j % 4 + 1) * P],
                   qt[:], True, True)
                pt = pt_pool.tile([P, 512], f32r, tag="pt")
                nc.scalar.activation(out=pt[:], in_=st[:], func=AF.Exp, scale=SCALE)
                nc.vector.scalar_tensor_tensor(
                    out=pt[:],
                    in0=qpos,
                    scalar=tpos_sb[:, j : j + 1],
                    in1=pt[:],
                    op0=mybir.AluOpType.is_ge,
                    op1=mybir.AluOpType.mult,
                )
                mm(ot[:], v_tiles[j // 4][:, j % 4, :], pt[:], j == 0, j == n_t - 1)
                mm(rs[:], ones[:], pt[:], j == 0, j == n_t - 1)

            nc.scalar.copy(rs_sb[0:1, qb * 512 : (qb + 1) * 512], rs[:])
            nc.vector.tensor_copy(ot_sb[:, qb * 512 : (qb + 1) * 512], ot[:])
            nc.sync.dma_start(
                out=ot_out[:, qb * 512 : (qb + 1) * 512],
                in_=ot_sb[:, qb * 512 : (qb + 1) * 512],
            )
            nc.scalar.dma_start(
                out=rs_out[:, qb * 512 : (qb + 1) * 512],
                in_=rs_sb[0:1, qb * 512 : (qb + 1) * 512],
            )

    nc.compile()
    return nc


_NC_CACHE = {}


def _get_nc():
    if "nc" not in _NC_CACHE:
        _NC_CACHE["nc"] = _build_nc()
    return _NC_CACHE["nc"]


def _get_runner():
    """Cached PJRT executable (same lowering as bass2jax.run_bass_via_pjrt,
    but the jitted function is built once and reused across calls)."""
    if "runner" in _NC_CACHE:
        return _NC_CACHE["runner"]

    import jax
    import jax.numpy as jnp
    from jax.sharding import Mesh, PartitionSpec
    from jax.experimental.shard_map import shard_map
    from concourse import bass2jax, mybir

    nc = _get_nc()
    bass2jax.install_neuronx_cc_hook()

    partition_name = nc.partition_id_tensor.name if nc.partition_id_tensor else None
    in_names, out_names, out_avals = [], [], []
    for alloc in nc.m.functions[0].allocations:
        if not isinstance(alloc, mybir.MemoryLocationSet):
            continue
        name = alloc.memorylocations[0].name
        if alloc.kind == "ExternalInput":
            if name != partition_name:
                in_names.append(name)
        elif alloc.kind == "ExternalOutput":
            out_names.append(name)
            out_avals.append(
                jax.core.ShapedArray(tuple(alloc.tensor_shape), mybir.dt.np(alloc.dtype))
            )
    n_params = len(in_names)
    n_outs = len(out_names)
    all_names = in_names + out_names
    if partition_name is not None:
        all_names = all_names + [partition_name]

    def _body(*args):
        operands = list(args)
        if partition_name is not None:
            operands.append(bass2jax.partition_id_tensor())
        outs = bass2jax._bass_exec_p.bind(
            *operands,
            out_avals=tuple(out_avals),
            in_names=tuple(all_names),
            out_names=tuple(out_names),
            lowering_input_output_aliases=(),
            sim_require_finite=True,
            sim_require_nnan=True,
            nc=nc,
        )
        return tuple(outs)

    devices = jax.devices()[:8]
    mesh = Mesh(__import__("numpy").asarray(devices), ("core",))
    sharded = jax.jit(
        shard_map(
            _body,
            mesh=mesh,
            in_specs=(PartitionSpec("core"),) * (n_params + n_outs),
            out_specs=(PartitionSpec("core"),) * n_outs,
            check_rep=False,
        ),
        donate_argnums=tuple(range(n_params, n_params + n_outs)),
        keep_unused=True,
    )
    runner = {
        "sharded": sharded,
        "in_names": in_names,
        "out_names": out_names,
        "out_avals": out_avals,
    }
    _NC_CACHE["runner"] = runner
    return runner


def _prep_in_concat(x, wq, bq, wk, bk, wv, bv):
    """Per-core in_maps, concatenated along axis 0 for shard_map."""
    x = np.asarray(x, dtype=np.float32)
    w = {
        "wq": np.asarray(wq, np.float32),
        "wk": np.asarray(wk, np.float32),
        "wv": np.asarray(wv, np.float32),
        "bq": np.asarray(bq, np.float32).reshape(P, 1),
        "bk": np.asarray(bk, np.float32).reshape(P, 1),
        "bv": np.asarray(bv, np.float32).reshape(P, 1),
    }
    if "perm" not in _NC_CACHE:
        _NC_CACHE["perm"] = [_role_perm(0), _role_perm(1)]
        tp, qp = [], []
        for role in (0, 1):
            perm = _NC_CACHE["perm"][role]
            tp.append(
                np.ascontiguousarray(
                    perm.reshape(NT, P).T.astype(np.float32)
                )
            )
            qp.append(
                np.ascontiguousarray(
                    np.concatenate([perm[0:512], perm[1024:1536]])
                    .astype(np.float32)
                    .reshape(1, 1024)
                )
            )
        _NC_CACHE["tpos"] = tp
        _NC_CACHE["qpos"] = qp
    perms = _NC_CACHE["perm"]

    runner = _get_runner()
    concat = {}
    concat["xp"] = np.concatenate(
        [x[c // 2][perms[c % 2]] for c in range(8)], axis=0
    )
    concat["tpos"] = np.concatenate([_NC_CACHE["tpos"][c % 2] for c in range(8)], axis=0)
    concat["ident"] = np.concatenate([np.eye(P, dtype=np.float32)] * 8, axis=0)
    concat["ones"] = np.ones((8 * P, 1), dtype=np.float32)
    concat["onesr"] = np.ones((8, P), dtype=np.float32)
    concat["qpos"] = np.concatenate([_NC_CACHE["qpos"][c % 2] for c in range(8)], axis=0)
    for n, v in w.items():
        concat[n] = np.concatenate([v] * 8, axis=0)
    return [concat[n] for n in runner["in_names"]]


def _run_concat(concat_in):
    runner = _get_runner()
    zeros = [
        np.zeros((8 * a.shape[0], *a.shape[1:]), a.dtype) for a in runner["out_avals"]
    ]
    out_arrs = runner["sharded"](*concat_in, *zeros)
    ot = np.asarray(out_arrs[runner["out_names"].index("ot")]).reshape(8, P, 1024)
    rs = np.asarray(out_arrs[runner["out_names"].index("rs")]).reshape(8, 1024)
    return ot, rs


def _assemble(ot, rs):
    perms = _NC_CACHE["perm"]
    out = np.empty((B, S, D), dtype=np.float32)
    for c in range(8):
        b, role = divmod(c, 2)
        perm = perms[role]
        for qb, qpos0 in ((0, 0), (1, 1024)):
            otT = ot[c][:, qb * 512 : (qb + 1) * 512]          # [D, 512]
            rsq = rs[c][qb * 512 : (qb + 1) * 512]             # [512]
            out[b, perm[qpos0 : qpos0 + 512]] = (otT / rsq[None, :]).T
    return out


def kernel(x, wq, bq, wk, bk, wv, bv):
    concat_in = _prep_in_concat(x, wq, bq, wk, bk, wv, bv)
    ot, rs = _run_concat(concat_in)
    return _assemble(ot, rs)


def bench(x, wq, bq, wk, bk, wv, bv, iters=20):
    """Per-launch wall time with device-resident inputs (upper bound on HW exec)."""
    import time

    import jax

    runner = _get_runner()
    concat_in = _prep_in_concat(x, wq, bq, wk, bk, wv, bv)
    dev_in = [jax.device_put(a) for a in concat_in]
    for a in dev_in:
        a.block_until_ready()
    times = []
    for _ in range(iters):
        zeros = [
            np.zeros((8 * a.shape[0], *a.shape[1:]), a.dtype)
            for a in runner["out_avals"]
        ]
        t0 = time.perf_counter()
        out = runner["sharded"](*dev_in, *zeros)
        for a in out:
            a.block_until_ready()
        times.append(time.perf_counter() - t0)
    return times


def bench_chain(x, wq, bq, wk, bk, wv, bv, ks=(2, 12), reps=6):
    """Marginal device time per kernel launch: chain k sequential launches
    inside one jit (data-dependent via the mask input), compare wall."""
    import time

    import jax
    import jax.numpy as jnp
    from jax.sharding import Mesh, PartitionSpec
    from jax.experimental.shard_map import shard_map
    from concourse import bass2jax

    runner = _get_runner()
    nc = _get_nc()
    partition_name = nc.partition_id_tensor.name if nc.partition_id_tensor else None
    in_names = runner["in_names"]
    out_names = runner["out_names"]
    out_avals = runner["out_avals"]
    all_names = in_names + out_names + ([partition_name] if partition_name else [])
    mask_idx = in_names.index("mask")

    concat_in = _prep_in_concat(x, wq, bq, wk, bk, wv, bv)
    dev_in = [jax.device_put(a) for a in concat_in]
    for a in dev_in:
        a.block_until_ready()

    import numpy as _np

    def make_fn(k):
        def _body(*args):
            ins = list(args[: len(in_names)])
            zero_sets = args[len(in_names) :]
            o = None
            outs = None
            for i in range(k):
                cur = list(ins)
                if o is not None:
                    bump = (o[0:1, 0:1] != o[0:1, 0:1]).astype(jnp.uint8)
                    cur[mask_idx] = cur[mask_idx] | bump
                operands = cur + list(zero_sets[i * len(out_names) : (i + 1) * len(out_names)])
                if partition_name is not None:
                    operands.append(bass2jax.partition_id_tensor())
                outs = bass2jax._bass_exec_p.bind(
                    *operands,
                    out_avals=tuple(out_avals),
                    in_names=tuple(all_names),
                    out_names=tuple(out_names),
                    lowering_input_output_aliases=(),
                    sim_require_finite=True,
                    sim_require_nnan=True,
                    nc=nc,
                )
                o = outs[0]
            return tuple(outs)

        n_z = k * len(out_names)
        devices = jax.devices()[:8]
        mesh = Mesh(_np.asarray(devices), ("core",))
        return jax.jit(
            shard_map(
                _body,
                mesh=mesh,
                in_specs=(PartitionSpec("core"),) * (len(in_names) + n_z),
                out_specs=(PartitionSpec("core"),) * len(out_names),
                check_rep=False,
            ),
            donate_argnums=tuple(range(len(in_names), len(in_names) + n_z)),
            keep_unused=True,
        )

    results = {}
    for k in ks:
        fn = make_fn(k)
        walls = []
        for _ in range(reps):
            zeros = [
                _np.zeros((8 * a.shape[0], *a.shape[1:]), a.dtype)
                for _ in range(k)
                for a in out_avals
            ]
            t0 = time.perf_counter()
            out = fn(*dev_in, *zeros)
            for a in out:
                a.block_until_ready()
            walls.append(time.perf_counter() - t0)
        results[k] = min(walls)
    k0, k1 = ks
    per_launch = (results[k1] - results[k0]) / (k1 - k0)
    return per_launch, results


# revision 36
# speedup vs baseline: 1606.8870x; 1.0047x over previous
"""Causal single-head attention (B=4, S=2048, E=1024, D=128) on 8 trn2 cores.

Sharding: 2 cores per batch. Each core computes the attention output for
1024 query rows of its batch. To keep one uniform SPMD program while
balancing the causal (triangular) work, the host permutes each batch's
rows per core role and ships a per-core 0/1 mask table:

  role 0: perm = [0:512 | 512:1024 | 1536:2048 | 1024:1536]
  role 1: perm = [512:1024 | 0:512 | 1024:1536 | 1536:2048]

Queries are the permuted positions [0,512) (q-block 0, key extent 1024)
and [1024,1536) (q-block 1, key extent 2048). Both roles then run the
exact same static program; causality (including wasted padded tiles) is
enforced by multiplying exp(scores) with the host-baked mask.

Per-core kernel (flow over transposed scores, fp32r matmuls):
  xT = PE-transpose of x (E on partitions)
  K^T/V^T/Q^T = w.T @ xT accumulated over 8 E-chunks; V re-transposed
  per q-block, per key tile j: st[t,s] = KT_j.T @ QT ; pt = exp(st*scale)
  pt *= mask ; rowacc += pt ; outT[D,s] += V_j.T @ pt
  rowsum via ones-matmul per 128-col chunk; out = transpose(outT) * 1/rowsum
"""

import math

import numpy as np

B, S, E, D = 4, 2048, 1024, 128
P = 128
EC = E // P          # 8 E-chunks
NT = S // P          # 16 key tiles
TB = S // 512        # 4 key blocks of 512
QB_NT = (8, 16)      # key-tile extent per q-block (padded, role-uniform)
N_MASK = QB_NT[0] + QB_NT[1]
SCALE = 1.0 / math.sqrt(D)

MM_DT = None  # set in _build_nc (float32r)


def _role_perm(role):
    a = np.arange
    if role == 0:
        blocks = [a(0, 512), a(512, 1024), a(1536, 2048), a(1024, 1536)]
    else:
        blocks = [a(512, 1024), a(0, 512), a(1024, 1536), a(1536, 2048)]
    return np.concatenate(blocks)


def _role_mask(role):
    """[128, N_MASK*512] u8: concat over (qb, j) of valid(t_pos, s_pos)."""
    perm = _role_perm(role)
    tiles = []
    for qb, qpos0 in ((0, 0), (1, 1024)):
        q_orig = perm[qpos0 : qpos0 + 512]
        for j in range(QB_NT[qb]):
            t_orig = perm[j * P : (j + 1) * P]
            tiles.append((t_orig[:, None] <= q_orig[None, :]).astype(np.uint8))
    return np.concatenate(tiles, axis=1)


def _build_nc():
    global MM_DT
    from contextlib import ExitStack

    import concourse.bass as bass
    import concourse.tile as tile
    from concourse import bacc, masks, mybir

    MM_DT = mybir.dt.float32r
    f32r = mybir.dt.float32r
    f32 = mybir.dt.float32
    u8 = mybir.dt.uint8
    AF = mybir.ActivationFunctionType

    nc = bacc.Bacc("TRN2", target_bir_lowering=False, debug=False)

    xp = nc.dram_tensor("xp", [S, E], f32r, kind="ExternalInput")
    w_in = {
        n: nc.dram_tensor(n, [E, D], f32r, kind="ExternalInput")
        for n in ("wq", "wk", "wv")
    }
    b_in = {
        n: nc.dram_tensor(n, [P, 1], f32, kind="ExternalInput")
        for n in ("bq", "bk", "bv")
    }
    tpos_in = nc.dram_tensor("tpos", [P, NT], f32, kind="ExternalInput")
    ident_in = nc.dram_tensor("ident", [P, P], f32r, kind="ExternalInput")
    ones_in = nc.dram_tensor("ones", [P, 1], f32r, kind="ExternalInput")
    onesr_in = nc.dram_tensor("onesr", [1, P], f32r, kind="ExternalInput")
    qpos_in = nc.dram_tensor("qpos", [1, 1024], f32r, kind="ExternalInput")
    ot_out = nc.dram_tensor("ot", [P, 1024], f32, kind="ExternalOutput")
    rs_out = nc.dram_tensor("rs", [1, 1024], f32, kind="ExternalOutput")

    def mm(out, lhsT, rhs, start, stop):
        nc.tensor.matmul(out, lhsT, rhs, start=start, stop=stop)

    with tile.TileContext(nc) as tc, ExitStack() as ctx:
        consts = ctx.enter_context(tc.tile_pool(name="consts", bufs=1))
        xn_pool = ctx.enter_context(tc.tile_pool(name="xn", bufs=32))
        xt_pool = ctx.enter_context(tc.tile_pool(name="xt", bufs=24))
        sb_pool = ctx.enter_context(tc.tile_pool(name="sb", bufs=2))
        pt_pool = ctx.enter_context(tc.tile_pool(name="pt", bufs=8))
        out_pool = ctx.enter_context(tc.tile_pool(name="outp", bufs=1))
        tr_psum = ctx.enter_context(tc.tile_pool(name="trp", bufs=2, space="PSUM"))
        st_psum = ctx.enter_context(tc.tile_pool(name="stp", bufs=2, space="PSUM"))
        proj_psum = ctx.enter_context(tc.tile_pool(name="pjp", bufs=3, space="PSUM"))
        sm_psum = ctx.enter_context(tc.tile_pool(name="smp", bufs=1, space="PSUM"))

        # identity first: the very first transposes need it
        ident_t = consts.tile([P, P], f32r, name="ident_t")
        nc.sync.dma_start(out=ident_t[:], in_=ident_in[:, :])
        ident = ident_t[:]

        # stage the first T-block's x rows before anything else so PE can
        # start transposing as early as possible
        xn_tiles = {}
        dmae = [nc.sync, nc.scalar]

        def load_xn(g):
            halves = []
            for h in range(2):
                t = xn_pool.tile([P, E // 2], f32r, tag="xn", name=f"xn_{g}_{h}")
                dmae[g % 2].dma_start(
                    out=t[:],
                    in_=xp[g * P : (g + 1) * P, h * (E // 2) : (h + 1) * (E // 2)],
                )
                halves.append(t)
            return halves

        # issue the low halves of the first block first: the e<4 transposes
        # depend only on them
        _pre = {tt: [None, None] for tt in range(4)}
        for h in range(2):
            for tt in range(4):
                t = xn_pool.tile([P, E // 2], f32r, tag="xn", name=f"xnp_{tt}_{h}")
                dmae[tt % 2].dma_start(
                    out=t[:],
                    in_=xp[tt * P : (tt + 1) * P, h * (E // 2) : (h + 1) * (E // 2)],
                )
                _pre[tt][h] = t
        for tt in range(4):
            xn_tiles[tt] = _pre[tt]

        ones = consts.tile([P, 1], f32r)
        nc.gpsimd.dma_start(out=ones[:], in_=ones_in[:, :])
        onesr = consts.tile([1, P], f32r)
        nc.sync.dma_start(out=onesr[:], in_=onesr_in[:, :])
        w_sb = {}
        for i, n in enumerate(("wk", "wv", "wq")):
            w_sb[n] = consts.tile([P, EC, D], f32r, name=f"w_{n}")
            dmae[i % 2].dma_start(
                out=w_sb[n][:], in_=w_in[n].rearrange("(c p) d -> p c d", p=P)
            )
        b_sb = {}
        for n in ("bq", "bk", "bv"):
            b_sb[n] = consts.tile([P, 1], f32, name=f"b_{n}")
            nc.gpsimd.dma_start(out=b_sb[n][:], in_=b_in[n][:, :])

        kt_tiles = {}   # per-tb K^T [D, 512]
        v_tiles = {}    # per-tb V natural [t_loc, 4, D]
        qt_tiles = {}   # per-qb Q^T [D, 512]
        tpos_sb = consts.tile([P, NT], f32)
        nc.gpsimd.dma_start(out=tpos_sb[:], in_=tpos_in[:, :])
        qpos1 = consts.tile([1, 1024], f32r)
        nc.sync.dma_start(out=qpos1[:], in_=qpos_in[:, :])
        qpos_sb = consts.tile([P, 1024], f32)
        for h in range(2):
            qb_ps = sm_psum.tile([P, 512], f32, tag="sm", name=f"qbps_{h}")
            nc.tensor.matmul(
                qb_ps[:], onesr[:], qpos1[0:1, h * 512 : (h + 1) * 512],
                start=True, stop=True,
            )
            nc.vector.tensor_copy(qpos_sb[:, h * 512 : (h + 1) * 512], qb_ps[:])

        # ---- phase 1: xT, projections ---------------------------------
        for tb in (0, 2, 1, 3):
            xn = []
            for tt in range(4):
                g = tb * 4 + tt
                if g in xn_tiles:
                    t = xn_tiles[g]
                else:
                    t = load_xn(g)
                xn.append(t)

            xt = []
            for e in range(EC):
                tp = tr_psum.tile([P, 512], f32r, tag="tr")
                for tt in range(4):
                    half = xn[tt][e // 4]
                    nc.tensor.matmul(
                        tp[:, tt * P : (tt + 1) * P],
                        half[:, (e % 4) * P : (e % 4 + 1) * P],
                        ident,
                        is_transpose=True,
                        start=(tt == 0),
                        stop=(tt == 3),
                    )
                xte = xt_pool.tile([P, 512], f32r, tag="xte", name=f"xt_{tb}_{e}")
                if e % 2 == 0:
                    nc.vector.tensor_copy(xte[:], tp[:])
                else:
                    nc.scalar.copy(xte[:], tp[:])
                xt.append(xte)

            # K^T
            pp = proj_psum.tile([P, 512], f32, tag="pj")
            for e in range(EC):
                mm(pp[:], w_sb["wk"][:, e, :], xt[e][:], e == 0, e == EC - 1)
            kt = consts.tile([P, 512], f32r, name=f"kt_{tb}")
            nc.scalar.activation(
                out=kt[:], in_=pp[:], func=AF.Identity, bias=b_sb["bk"][:]
            )
            kt_tiles[tb] = kt

            # V^T -> V natural
            pp = proj_psum.tile([P, 512], f32, tag="pj")
            for e in range(EC):
                mm(pp[:], w_sb["wv"][:, e, :], xt[e][:], e == 0, e == EC - 1)
            vt = sb_pool.tile([P, 512], f32r, tag="vt")
            nc.scalar.activation(
                out=vt[:], in_=pp[:], func=AF.Identity, bias=b_sb["bv"][:]
            )
            vp = tr_psum.tile([P, 512], f32r, tag="tr")
            for tt in range(4):
                nc.tensor.matmul(
                    vp[:, tt * P : (tt + 1) * P],
                    vt[:, tt * P : (tt + 1) * P],
                    ident,
                    is_transpose=True,
                    start=(tt == 0),
                    stop=(tt == 3),
                )
            v = consts.tile([P, 4, D], f32r, name=f"v_{tb}")
            nc.vector.tensor_copy(v[:], vp[:])
            v_tiles[tb] = v

            # Q^T (q-block 0 lives at pos [0,512) = tb0; q-block 1 at tb2)
            if tb in (0, 2):
                qb = 0 if tb == 0 else 1
                pp = proj_psum.tile([P, 512], f32, tag="pj")
                for e in range(EC):
                    mm(pp[:], w_sb["wq"][:, e, :], xt[e][:], e == 0, e == EC - 1)
                qt = consts.tile([P, 512], f32r, name=f"qt_{qb}")
                nc.scalar.activation(
                    out=qt[:], in_=pp[:], func=AF.Identity, bias=b_sb["bq"][:]
                )
                qt_tiles[qb] = qt

        # ---- phase 2: attention ---------------------------------------
        ot_sb = out_pool.tile([P, 1024], f32)
        rs_sb = out_pool.tile([1, 1024], f32)
        for qb in (0, 1):
            n_t = QB_NT[qb]
            qt = qt_tiles[qb]
            qpos = qpos_sb[:, qb * 512 : (qb + 1) * 512]
            ot = proj_psum.tile([P, 512], f32, tag="pj")
            rs = sm_psum.tile([1, 512], f32, tag="sm")
            for j in range(n_t):
                st = st_psum.tile([P, 512], f32, tag="st")
                mm(st[:], kt_tiles[j // 4][:, (j % 4) * P : (j % 4 + 1) * P],
                   qt[:], True, True)
                pt = pt_pool.tile([P, 512], f32r, tag="pt")
                nc.scalar.activation(out=pt[:], in_=st[:], func=AF.Exp, scale=SCALE)
                nc.vector.scalar_tensor_tensor(
                    out=pt[:],
                    in0=qpos,
                    scalar=tpos_sb[:, j : j + 1],
                    in1=pt[:],
                    op0=mybir.AluOpType.is_ge,
                    op1=mybir.AluOpType.mult,
                )
                mm(ot[:], v_tiles[j // 4][:, j % 4, :], pt[:], j == 0, j == n_t - 1)
                mm(rs[:], ones[:], pt[:], j == 0, j == n_t - 1)

            nc.scalar.copy(rs_sb[0:1, qb * 512 : (qb + 1) * 512], rs[:])
            nc.vector.tensor_copy(ot_sb[:, qb * 512 : (qb + 1) * 512], ot[:])
            nc.sync.dma_start(
                out=ot_out[:, qb * 512 : (qb + 1) * 512],
                in_=ot_sb[:, qb * 512 : (qb + 1) * 512],
            )
            nc.scalar.dma_start(
                out=rs_out[:, qb * 512 : (qb + 1) * 512],
                in_=rs_sb[0:1, qb * 512 : (qb + 1) * 512],
            )

    nc.compile()
    return nc


_NC_CACHE = {}


def _get_nc():
    if "nc" not in _NC_CACHE:
        _NC_CACHE["nc"] = _build_nc()
    return _NC_CACHE["nc"]


def _get_runner():
    """Cached PJRT executable (same lowering as bass2jax.run_bass_via_pjrt,
    but the jitted function is built once and reused across calls)."""
    if "runner" in _NC_CACHE:
        return _NC_CACHE["runner"]

    import jax
    import jax.numpy as jnp
    from jax.sharding import Mesh, PartitionSpec
    from jax.experimental.shard_map import shard_map
    from concourse import bass2jax, mybir

    nc = _get_nc()
    bass2jax.install_neuronx_cc_hook()

    partition_name = nc.partition_id_tensor.name if nc.partition_id_tensor else None
    in_names, out_names, out_avals = [], [], []
    for alloc in nc.m.functions[0].allocations:
        if not isinstance(alloc, mybir.MemoryLocationSet):
            continue
        name = alloc.memorylocations[0].name
        if alloc.kind == "ExternalInput":
            if name != partition_name:
                in_names.append(name)
        elif alloc.kind == "ExternalOutput":
            out_names.append(name)
            out_avals.append(
                jax.core.ShapedArray(tuple(alloc.tensor_shape), mybir.dt.np(alloc.dtype))
            )
    n_params = len(in_names)
    n_outs = len(out_names)
    all_names = in_names + out_names
    if partition_name is not None:
        all_names = all_names + [partition_name]

    def _body(*args):
        operands = list(args)
        if partition_name is not None:
            operands.append(bass2jax.partition_id_tensor())
        outs = bass2jax._bass_exec_p.bind(
            *operands,
            out_avals=tuple(out_avals),
            in_names=tuple(all_names),
            out_names=tuple(out_names),
            lowering_input_output_aliases=(),
            sim_require_finite=True,
            sim_require_nnan=True,
            nc=nc,
        )
        return tuple(outs)

    devices = jax.devices()[:8]
    mesh = Mesh(__import__("numpy").asarray(devices), ("core",))
    sharded = jax.jit(
        shard_map(
            _body,
            mesh=mesh,
            in_specs=(PartitionSpec("core"),) * (n_params + n_outs),
            out_specs=(PartitionSpec("core"),) * n_outs,
            check_rep=False,
        ),
        donate_argnums=tuple(range(n_params, n_params + n_outs)),
        keep_unused=True,
    )
    runner = {
        "sharded": sharded,
        "in_names": in_names,
        "out_names": out_names,
        "out_avals": out_avals,
    }
    _NC_CACHE["runner"] = runner
    return runner


def _prep_in_concat(x, wq, bq, wk, bk, wv, bv):
    """Per-core in_maps, concatenated along axis 0 for shard_map."""
    x = np.asarray(x, dtype=np.float32)
    w = {
        "wq": np.asarray(wq, np.float32),
        "wk": np.asarray(wk, np.float32),
        "wv": np.asarray(wv, np.float32),
        "bq": np.asarray(bq, np.float32).reshape(P, 1),
        "bk": np.asarray(bk, np.float32).reshape(P, 1),
        "bv": np.asarray(bv, np.float32).reshape(P, 1),
    }
    if "perm" not in _NC_CACHE:
        _NC_CACHE["perm"] = [_role_perm(0), _role_perm(1)]
        tp, qp = [], []
        for role in (0, 1):
            perm = _NC_CACHE["perm"][role]
            tp.append(
                np.ascontiguousarray(
                    perm.reshape(NT, P).T.astype(np.float32)
                )
            )
            qp.append(
                np.ascontiguousarray(
                    np.concatenate([perm[0:512], perm[1024:1536]])
                    .astype(np.float32)
                    .reshape(1, 1024)
                )
            )
        _NC_CACHE["tpos"] = tp
        _NC_CACHE["qpos"] = qp
    perms = _NC_CACHE["perm"]

    runner = _get_runner()
    concat = {}
    concat["xp"] = np.concatenate(
        [x[c // 2][perms[c % 2]] for c in range(8)], axis=0
    )
    concat["tpos"] = np.concatenate([_NC_CACHE["tpos"][c % 2] for c in range(8)], axis=0)
    concat["ident"] = np.concatenate([np.eye(P, dtype=np.float32)] * 8, axis=0)
    concat["ones"] = np.ones((8 * P, 1), dtype=np.float32)
    concat["onesr"] = np.ones((8, P), dtype=np.float32)
    concat["qpos"] = np.concatenate([_NC_CACHE["qpos"][c % 2] for c in range(8)], axis=0)
    for n, v in w.items():
        concat[n] = np.concatenate([v] * 8, axis=0)
    return [concat[n] for n in runner["in_names"]]


def _run_concat(concat_in):
    runner = _get_runner()
    zeros = [
        np.zeros((8 * a.shape[0], *a.shape[1:]), a.dtype) for a in runner["out_avals"]
    ]
    out_arrs = runner["sharded"](*concat_in, *zeros)
    ot = np.asarray(out_arrs[runner["out_names"].index("ot")]).reshape(8, P, 1024)
    rs = np.asarray(out_arrs[runner["out_names"].index("rs")]).reshape(8, 1024)
    return ot, rs


def _assemble(ot, rs):
    perms = _NC_CACHE["perm"]
    out = np.empty((B, S, D), dtype=np.float32)
    for c in range(8):
        b, role = divmod(c, 2)
        perm = perms[role]
        for qb, qpos0 in ((0, 0), (1, 1024)):
            otT = ot[c][:, qb * 512 : (qb + 1) * 512]          # [D, 512]
            rsq = rs[c][qb * 512 : (qb + 1) * 512]             # [512]
            out[b, perm[qpos0 : qpos0 + 512]] = (otT / rsq[None, :]).T
    return out


def kernel(x, wq, bq, wk, bk, wv, bv):
    concat_in = _prep_in_concat(x, wq, bq, wk, bk, wv, bv)
    ot, rs = _run_concat(concat_in)
    return _assemble(ot, rs)


def bench(x, wq, bq, wk, bk, wv, bv, iters=20):
    """Per-launch wall time with device-resident inputs (upper bound on HW exec)."""
    import time

    import jax

    runner = _get_runner()
    concat_in = _prep_in_concat(x, wq, bq, wk, bk, wv, bv)
    dev_in = [jax.device_put(a) for a in concat_in]
    for a in dev_in:
        a.block_until_ready()
    times = []
    for _ in range(iters):
        zeros = [
            np.zeros((8 * a.shape[0], *a.shape[1:]), a.dtype)
            for a in runner["out_avals"]
        ]
        t0 = time.perf_counter()
        out = runner["sharded"](*dev_in, *zeros)
        for a in out:
            a.block_until_ready()
        times.append(time.perf_counter() - t0)
    return times


def bench_chain(x, wq, bq, wk, bk, wv, bv, ks=(2, 12), reps=6):
    """Marginal device time per kernel launch: chain k sequential launches
    inside one jit (data-dependent via the mask input), compare wall."""
    import time

    import jax
    import jax.numpy as jnp
    from jax.sharding import Mesh, PartitionSpec
    from jax.experimental.shard_map import shard_map
    from concourse import bass2jax

    runner = _get_runner()
    nc = _get_nc()
    partition_name = nc.partition_id_tensor.name if nc.partition_id_tensor else None
    in_names = runner["in_names"]
    out_names = runner["out_names"]
    out_avals = runner["out_avals"]
    all_names = in_names + out_names + ([partition_name] if partition_name else [])
    mask_idx = in_names.index("mask")

    concat_in = _prep_in_concat(x, wq, bq, wk, bk, wv, bv)
    dev_in = [jax.device_put(a) for a in concat_in]
    for a in dev_in:
        a.block_until_ready()

    import numpy as _np

    def make_fn(k):
        def _body(*args):
            ins = list(args[: len(in_names)])
            zero_sets = args[len(in_names) :]
            o = None
            outs = None
            for i in range(k):
                cur = list(ins)
                if o is not None:
                    bump = (o[0:1, 0:1] != o[0:1, 0:1]).astype(jnp.uint8)
                    cur[mask_idx] = cur[mask_idx] | bump
                operands = cur + list(zero_sets[i * len(out_names) : (i + 1) * len(out_names)])
                if partition_name is not None:
                    operands.append(bass2jax.partition_id_tensor())
                outs = bass2jax._bass_exec_p.bind(
                    *operands,
                    out_avals=tuple(out_avals),
                    in_names=tuple(all_names),
                    out_names=tuple(out_names),
                    lowering_input_output_aliases=(),
                    sim_require_finite=True,
                    sim_require_nnan=True,
                    nc=nc,
                )
                o = outs[0]
            return tuple(outs)

        n_z = k * len(out_names)
        devices = jax.devices()[:8]
        mesh = Mesh(_np.asarray(devices), ("core",))
        return jax.jit(
            shard_map(
                _body,
                mesh=mesh,
                in_specs=(PartitionSpec("core"),) * (len(in_names) + n_z),
                out_specs=(PartitionSpec("core"),) * len(out_names),
                check_rep=False,
            ),
            donate_argnums=tuple(range(len(in_names), len(in_names) + n_z)),
            keep_unused=True,
        )

    results = {}
    for k in ks:
        fn = make_fn(k)
        walls = []
        for _ in range(reps):
            zeros = [
                _np.zeros((8 * a.shape[0], *a.shape[1:]), a.dtype)
                for _ in range(k)
                for a in out_avals
            ]
            t0 = time.perf_counter()
            out = fn(*dev_in, *zeros)
            for a in out:
                a.block_until_ready()
            walls.append(time.perf_counter() - t0)
        results[k] = min(walls)
    k0, k1 = ks
    per_launch = (results[k1] - results[k0]) / (k1 - k0)
    return per_launch, results
